# revision 66
# baseline (speedup 1.0000x reference)
"""CharCNN token embedder (ELMo-style) on 8 Trainium2 NeuronCores.

Data-parallel over the 4096 = 16*256 tokens (512 per core). Weights replicated.

Per-core pipeline:
  1. dma_gather (transpose mode) pulls char-embedding rows (padded to 256B)
     into feature-major layout X[d, (t', n)] for t' in [0,56), n in [0,512).
  2. 7 shifted SBUF->SBUF copies build the patch matrix Xs[(k,d), (t, n)]
     (112 x 25600) = im2col for a width-7 window (weights zero-padded).
  3. Conv = bf16 matmuls with K=112: per 128-channel tile, one matmul per
     valid position t (N=512 tokens), PSUM rounds of 4 banks.
  4. Max-pool over positions per 4-bank PSUM round: ACT copies banks 1,3
     to SBUF, DVE pair-maxes them against banks 0,2 (one PSUM operand per
     DVE op -- a walrus requirement), then folds into two alternating
     accumulator chains; relu+bias at tile finalize; h written both bf16
     and fp8 (x64 scale).
  5. 2 highway layers in fp8e4 DoubleRow split precision: W ~ (Whi+Wlo)/s,
     K=256 per DR matmul, 16 DR matmuls per (j, half) accumulate one PSUM
     bank; ACT applies 1/(s*64) scale (per-partition AP) + bias + relu /
     sigmoid; DVE does the highway gating in bf16 4x mode.
  6. Projection to 512 in bf16, bias, PE-transpose to token-major, DMA out.
"""

from contextlib import ExitStack

import numpy as np
import ml_dtypes

import concourse.bass as bass
import concourse.mybir as mybir
import concourse.tile as tile
from concourse import bacc
from concourse.bass_utils import run_bass_kernel_spmd
from concourse.vector_clock import ScopedClock

# ---------------------------------------------------------------- constants
B, S, L = 16, 256, 50
CHAR_DIM = 16
CHAR_VOCAB = 262
PAD_V = 264            # table rows (262 real + 1 zero pad row + 1 spare)
ZERO_ROW = 262
FILTERS = [(1, 32), (2, 32), (3, 64), (4, 128), (5, 256), (6, 512), (7, 1024)]
N_FILTERS = 2048
PROJ_DIM = 512
N_CORES = 8
NTOK = B * S                  # 4096
TOK = NTOK // N_CORES         # 512 tokens per core
NPOS = 50                     # conv output positions computed
NI = TOK * NPOS               # gather indices per core = 25600
GATHER_CHUNK = 6400           # indices per dma_gather (descriptor-ring safe)
FREE = TOK * NPOS             # X_stack free size = 25600
KDIM = 112                    # 7 taps * 16 dims
SH = 64.0                     # fp8 activation scale for h0/h1 (absmax ~0.65)

# per 128-channel tile: full-row position count + (t, row_hi) row-tails
CH_TILES = [{"t_main": 48, "tails": [(48, 64), (49, 32)]}]
CH_TILES.append({"t_main": 47, "tails": []})      # w4
for _ in range(2):
    CH_TILES.append({"t_main": 46, "tails": []})  # w5
for _ in range(4):
    CH_TILES.append({"t_main": 45, "tails": []})  # w6
for _ in range(8):
    CH_TILES.append({"t_main": 44, "tails": []})  # w7
R_POS = 4

BF16 = mybir.dt.bfloat16
FP32 = mybir.dt.float32
F8 = mybir.dt.float8e4
MAX_OP = mybir.AluOpType.max

_MAX_WAITS_PER_INST = 1


def _patched_drain_and_barrier(self, tick_clock, wait_clock):
    # The walrus build in this container rejects CTRL instructions carrying
    # more than one sem wait; spread the kernel-tail drain waits over NOPs.
    nc = self.nc
    carrier = nc.sync.nop()
    wait_clock.add_sem_waits(carrier.ins, ScopedClock({None: tick_clock.global_clock}))
    si = carrier.ins.sync_info
    waits = list(si.on_wait) if si is not None and si.on_wait else []
    if len(waits) > _MAX_WAITS_PER_INST:
        carrier.ins.sync_info = mybir.SyncInfo(
            on_wait=waits[:_MAX_WAITS_PER_INST],
            on_update=list(si.on_update) if si.on_update else [])
        for i in range(_MAX_WAITS_PER_INST, len(waits), _MAX_WAITS_PER_INST):
            extra = nc.sync.nop()
            extra.ins.sync_info = mybir.SyncInfo(
                on_wait=waits[i:i + _MAX_WAITS_PER_INST], on_update=[])
    nc.sync.drain()
    nc.all_engine_barrier()
    assert self.sems is not None
    popped = nc._tile_sem_poison_stack.pop()
    assert popped is self._sem_poison
    nc.clear_and_free_semaphores(list(self.sems.allocated().values()))
    nc.all_engine_barrier()


tile.TileContext._drain_and_barrier = _patched_drain_and_barrier


class PoolSched:
    """Greedy engine-load balancer for the conv max-pool stage.

    Cost constants are exact TimelineSim per-instruction engine times."""

    def __init__(self, nc, spool):
        self.nc = nc
        self.spool = spool
        self.est = {"dve": 0.0, "act": 0.0, "pool": 0.0}

    def _pick(self, options):
        """options: list of (key, {eng: cost}). Pick min resulting max-load."""
        best, bestv = None, None
        for key, costs in options:
            peak = max(self.est[e] + costs.get(e, 0.0) for e in self.est)
            if bestv is None or peak < bestv:
                best, bestv = (key, costs), peak
        for e, c in best[1].items():
            self.est[e] += c
        return best[0]

    def fold(self, acc_ap, in_ap, nelem):
        """acc = max(acc, in); DVE only (walrus rejects TT on gpsimd)."""
        self.est["dve"] += {512: 328.0, 1024: 594.0, 2048: 1127.0}[nelem]
        self.nc.vector.tensor_tensor(
            out=acc_ap, in0=acc_ap, in1=in_ap, op=MAX_OP)

    def round(self, P, nt, st, ridx):
        """Drain one PSUM round (nt banks) into an independent chain.

        walrus allows at most ONE PSUM operand per DVE op, so drains are
        either a fused TT(PSUM, acc_sbuf)->acc (quad chains D1/D2 on DVE)
        or an ACT copy + SBUF-side fold (quad chain C).  acc16 slots:
        0:4 = D1, 4:8 = D2, 8:12 = C."""
        nc, spool = self.nc, self.spool
        acc = st["acc16"]
        if nt == 4:
            path = self._pick([("dve", {"dve": 2258.0}),
                               ("act", {"act": 1992.0, "dve": 0.0})])
            if path == "dve":
                k = st["dsel"]
                st["dsel"] ^= 1
                sl = 4 * k
                if st["first"][k]:
                    nc.vector.tensor_copy(out=acc[:, sl:sl + 4, :], in_=P[:, 0:4, :])
                    st["first"][k] = False
                else:
                    nc.vector.tensor_tensor(
                        out=acc[:, sl:sl + 4, :], in0=P[:, 0:4, :],
                        in1=acc[:, sl:sl + 4, :], op=MAX_OP)
            else:
                k = st["csel"]
                st["csel"] ^= 1
                sl = 8 + 4 * k
                if st["first"][2 + k]:
                    nc.scalar.copy(out=acc[:, sl:sl + 4, :], in_=P[:, 0:4, :])
                    st["first"][2 + k] = False
                else:
                    t = spool.tile([128, 4, TOK], BF16, tag="t4", name="tdr4")
                    nc.scalar.copy(out=t[:], in_=P[:, 0:4, :])
                    self.fold(acc[:, sl:sl + 4, :], t[:], 2048)
        else:
            # tail rounds (1-3 banks): fused max into chain D1's prefix
            if st["first"][0]:
                nc.vector.tensor_copy(out=acc[:, 0:nt, :], in_=P[:, 0:nt, :])
                st["first"][0] = False
                # remaining D1 slots stay virgin: seed them too
                if nt < 4:
                    nc.vector.tensor_copy(out=acc[:, nt:4, :],
                                          in_=P[:, 0:4 - nt, :])
            else:
                nc.vector.tensor_tensor(
                    out=acc[:, 0:nt, :], in0=P[:, 0:nt, :],
                    in1=acc[:, 0:nt, :], op=MAX_OP)
            self.est["dve"] += {1: 658.0, 2: 1192.0, 3: 1725.0}[nt]

    def finalize(self, st, h_b, h_f, bias_ap, i):
        nc, spool = self.nc, self.spool
        acc = st["acc16"]
        # combine quad chains (DVE)
        srcs = [acc[:, 0:4, :]]
        for k in (1, 2, 3):
            if not st["first"][k]:
                srcs.append(acc[:, 4 * k:4 * k + 4, :])
        while len(srcs) > 1:
            b = srcs.pop()
            a = srcs[-1]
            self.est["dve"] += 1127.0
            nc.vector.tensor_tensor(out=a, in0=a, in1=b, op=MAX_OP)
        m2 = spool.tile([128, 2, TOK], BF16, tag="t2")
        nc.vector.tensor_tensor(
            out=m2[:], in0=acc[:, 0:2, :], in1=acc[:, 2:4, :], op=MAX_OP)
        nc.vector.tensor_tensor(
            out=m2[:, 0:1, :], in0=m2[:, 0:1, :], in1=m2[:, 1:2, :], op=MAX_OP)
        self.est["dve"] += 594.0 + 328.0
        nc.scalar.activation(
            out=h_b[:, i, :], in_=m2[:, 0, :],
            func=mybir.ActivationFunctionType.Relu,
            bias=bias_ap, scale=1.0)
        self.est["act"] += 712.0
        if h_f is not None:
            self.est["dve"] += 327.0
            nc.vector.tensor_scalar_mul(
                out=h_f[:, i, :], in0=h_b[:, i, :], scalar1=SH)


# ---------------------------------------------------------------- device IR
def build_module():
    nc = bacc.Bacc()
    SIdx = NI // 16  # 1792 int16 columns

    table = nc.dram_tensor("table", [PAD_V, 128], BF16, kind="ExternalInput")
    idx = nc.dram_tensor("idx", [128, SIdx], mybir.dt.int16, kind="ExternalInput")
    wconv = nc.dram_tensor("wconv", [KDIM, N_FILTERS], BF16, kind="ExternalInput")
    bconv = nc.dram_tensor("bconv", [128, 16], FP32, kind="ExternalInput")
    # fp8 split highway weights: [l, j, p, (hl, half, c, s2, o)] flattened
    whw8 = nc.dram_tensor("whw8", [2, 16, 128, 8192], F8, kind="ExternalInput")
    bhw = nc.dram_tensor("bhw", [128, 2, 16, 2], FP32, kind="ExternalInput")
    schw = nc.dram_tensor("schw", [128, 2], FP32, kind="ExternalInput")
    wproj = nc.dram_tensor("wproj", [4, 128, 16, 128], BF16, kind="ExternalInput")
    bproj = nc.dram_tensor("bproj", [128, 4], FP32, kind="ExternalInput")
    ident = nc.dram_tensor("ident", [128, 128], FP32, kind="ExternalInput")
    out = nc.dram_tensor("out", [TOK, PROJ_DIM], FP32, kind="ExternalOutput")

    with tile.TileContext(nc) as tc:
        with (
            tc.tile_pool(name="consts", bufs=1) as cpool,
            tc.tile_pool(name="hbuf", bufs=2) as hpool,
            tc.tile_pool(name="wstream", bufs=3) as wpool,
            tc.tile_pool(name="wproj", bufs=2) as wppool,
            tc.tile_pool(name="small", bufs=2) as spool,
        ):
            # xs lives only through the conv phase; its scoped pool frees
            # 50KB for the highway weight stream
            with tc.tile_pool(name="xsp", bufs=1) as xspool:
                # ---- 1+2. gather char embeddings chunk-by-chunk into a
                # small ring buffer; scatter each chunk into the 7 tap bands
                # of the patch matrix as it lands (pipelines gather vs copy).
                with tc.tile_pool(name="gather", bufs=2) as gpool:
                    # idx first on the DMA queue: it gates the gather DGE
                    idx_t = gpool.tile([128, SIdx], mybir.dt.int16, tag="idx")
                    nc.sync.dma_start(out=idx_t[:], in_=idx[:])
                    wconv_t = cpool.tile([KDIM, N_FILTERS], BF16)
                    nc.sync.dma_start(out=wconv_t[:], in_=wconv[:])
                    bconv_t = cpool.tile([128, 16], FP32)
                    nc.sync.dma_start(out=bconv_t[:], in_=bconv[:])
                    bhw_t = cpool.tile([128, 2, 16, 2], FP32)
                    nc.sync.dma_start(out=bhw_t[:], in_=bhw[:])
                    schw_t = cpool.tile([128, 2], FP32)
                    nc.sync.dma_start(out=schw_t[:], in_=schw[:])
                    bproj_t = cpool.tile([128, 4], FP32)
                    nc.sync.dma_start(out=bproj_t[:], in_=bproj[:])
                    ident_t = cpool.tile([128, 128], FP32)
                    nc.sync.dma_start(out=ident_t[:], in_=ident[:])

                    xs = xspool.tile([KDIM, FREE], BF16)
                    # tap band k's last 512k cols correspond to char
                    # positions >= 50 and must read as zero.  One
                    # partition-0-based memset over the union region; the
                    # tap copies below rewrite the valid parts.
                    nc.vector.memset(xs[:, FREE - 512 * 6:], 0)

                    for r in range(NI // GATHER_CHUNK):
                        o = r * GATHER_CHUNK
                        xgc = gpool.tile([128, 1, GATHER_CHUNK], BF16, tag="xgc")
                        nc.gpsimd.dma_gather(
                            out_ap=xgc[:],
                            in_ap=table[:],
                            idxs_ap=idx_t[:, o // 16:(o + GATHER_CHUNK) // 16],
                            num_idxs=GATHER_CHUNK,
                            num_idxs_reg=GATHER_CHUNK,
                            elem_size=128,
                            transpose=True,
                            single_packet=False,
                        )
                        for k in range(7):
                            # xs[16k+d, c] = xg[d, c + 512k]; chunk covers
                            # xg cols [o, o+CHUNK)
                            lo = max(0, o - 512 * k)
                            hi = min(FREE - 512 * k, o + GATHER_CHUNK - 512 * k)
                            if lo >= hi:
                                continue
                            nc.sync.dma_start(
                                out=xs[16 * k:16 * (k + 1), lo:hi],
                                in_=xgc[0:16, 0, lo + 512 * k - o:hi + 512 * k - o],
                            )

                # ---- 3+4. conv + max pool + relu -> h tiles (bf16 + fp8)
                # 2-bank PSUM rounds in a 4-deep ring: drain latency hides
                # behind 3 rounds of PE lookahead.  Per round, greedy pick:
                #  - A2: one 2-bank ACT copy + DVE pair-fold   (ACT 1138/DVE 594)
                #  - D2: ACT copies bank1, DVE maxes bank0 vs it, slab-fold
                #                                              (ACT 712/DVE 986)
                # Four slab chains in acc4 keep folds off the critical path.
                with tc.tile_pool(name="convp", bufs=4, space="PSUM") as convp:
                    h0_b = hpool.tile([128, 16, TOK], BF16, tag="hb")
                    h0_f = hpool.tile([128, 16, TOK], F8, tag="hf")
                    est = {"dve": 0.0, "act": 0.0}
                    for i, spec in enumerate(CH_TILES):
                        lhsT = wconv_t[:, 128 * i:128 * (i + 1)]
                        acc4 = spool.tile([128, 4, TOK], BF16, tag="acc4")
                        first4 = [True, True, True, True]
                        t_cnt = spec["t_main"]
                        ridx = 0
                        t0 = 0
                        while t0 < t_cnt:
                            nt = min(2, t_cnt - t0)
                            P = convp.tile([128, 2, TOK], FP32, tag="ps")
                            for r in range(nt):
                                t = t0 + r
                                nc.tensor.matmul(
                                    out=P[:, r, :], lhsT=lhsT,
                                    rhs=xs[:, TOK * t:TOK * (t + 1)],
                                    start=True, stop=True)
                            if nt == 2:
                                pair = ridx % 2
                                sl = 2 * pair
                                slot = ridx % 4
                                a_first = first4[sl] and first4[sl + 1]
                                # strict D,A,D cycle (f_d=2/3 balances DVE/ACT)
                                # after two forced A2 seed rounds; determinism
                                # avoids greedy burstiness -> steadier pipeline
                                use_a2 = (ridx < 2) or (ridx % 3 == 1)
                                if use_a2:
                                    # A2: 2-bank ACT copy (forced for the
                                    # first two rounds to seed all 4 slots)
                                    est["act"] += 1138.0
                                    if a_first:
                                        nc.scalar.copy(out=acc4[:, sl:sl + 2, :],
                                                       in_=P[:, 0:2, :])
                                        first4[sl] = first4[sl + 1] = False
                                    else:
                                        t2 = spool.tile([128, 2, TOK], BF16,
                                                        tag="sp", name="t2a")
                                        nc.scalar.copy(out=t2[:], in_=P[:, 0:2, :])
                                        nc.vector.tensor_tensor(
                                            out=acc4[:, sl:sl + 2, :],
                                            in0=acc4[:, sl:sl + 2, :],
                                            in1=t2[:], op=MAX_OP)
                                        est["dve"] += 594.0
                                else:
                                    # D2: bank1 via ACT, bank0 via DVE max
                                    c = spool.tile([128, TOK], BF16, tag="c0",
                                                   name="cd")
                                    nc.scalar.copy(out=c[:], in_=P[:, 1, :])
                                    est["act"] += 712.0
                                    if first4[slot]:
                                        nc.vector.tensor_tensor(
                                            out=acc4[:, slot:slot + 1, :],
                                            in0=P[:, 0:1, :],
                                            in1=c[:].unsqueeze(1), op=MAX_OP)
                                        first4[slot] = False
                                        est["dve"] += 658.0
                                    else:
                                        tl = spool.tile([128, 2, TOK], BF16,
                                                        tag="sp", name="tld")
                                        nc.vector.tensor_tensor(
                                            out=tl[:, 0:1, :], in0=P[:, 0:1, :],
                                            in1=c[:].unsqueeze(1), op=MAX_OP)
                                        nc.vector.tensor_tensor(
                                            out=acc4[:, slot:slot + 1, :],
                                            in0=acc4[:, slot:slot + 1, :],
                                            in1=tl[:, 0:1, :], op=MAX_OP)
                                        est["dve"] += 986.0
                            else:
                                # single-bank tail (odd t_cnt)
                                slot = ridx % 4
                                tl = spool.tile([128, 2, TOK], BF16, tag="sp",
                                                name="tl1")
                                nc.scalar.copy(out=tl[:, 0:1, :], in_=P[:, 0:1, :])
                                est["act"] += 712.0
                                nc.vector.tensor_tensor(
                                    out=acc4[:, slot:slot + 1, :],
                                    in0=acc4[:, slot:slot + 1, :],
                                    in1=tl[:, 0:1, :], op=MAX_OP)
                                est["dve"] += 328.0
                            ridx += 1
                            t0 += nt
                        # row-tails (tile 0): positions valid for a row subset
                        if spec["tails"]:
                            P = convp.tile([128, 2, TOK], FP32, tag="ps")
                            for r, (t, hi) in enumerate(spec["tails"]):
                                nc.tensor.matmul(
                                    out=P[:, r, :], lhsT=lhsT,
                                    rhs=xs[:, TOK * t:TOK * (t + 1)],
                                    start=True, stop=True)
                            for r, (t, hi) in enumerate(spec["tails"]):
                                tl = spool.tile([128, 2, TOK], BF16, tag="sp",
                                                name="tlr")
                                nc.scalar.copy(out=tl[:, 0:1, :],
                                               in_=P[:, r:r + 1, :])
                                nc.vector.tensor_tensor(
                                    out=acc4[0:hi, 0:1, :],
                                    in0=acc4[0:hi, 0:1, :],
                                    in1=tl[0:hi, 0:1, :], op=MAX_OP)
                                est["act"] += 712.0
                                est["dve"] += 328.0
                        m2 = spool.tile([128, 2, TOK], BF16, tag="m2")
                        nc.vector.tensor_tensor(
                            out=m2[:], in0=acc4[:, 0:2, :], in1=acc4[:, 2:4, :],
                            op=MAX_OP)
                        v = spool.tile([128, TOK], BF16, tag="vv")
                        nc.vector.tensor_tensor(
                            out=v[:], in0=m2[:, 0, :], in1=m2[:, 1, :],
                            op=MAX_OP)
                        est["dve"] += 594.0 + 328.0
                        nc.scalar.activation(
                            out=h0_b[:, i, :], in_=v[:],
                            func=mybir.ActivationFunctionType.Relu,
                            bias=bconv_t[:, i:i + 1], scale=1.0)
                        est["act"] += 712.0
                        nc.vector.tensor_scalar_mul(
                            out=h0_f[:, i, :], in0=h0_b[:, i, :], scalar1=SH)
                        est["dve"] += 327.0

            # ---- 5. highway layers: fp8 DoubleRow split precision
            DR = mybir.MatmulPerfMode.DoubleRow
            stack = ExitStack()
            hwp = stack.enter_context(tc.tile_pool(name="hwp", bufs=2, space="PSUM"))
            pjp = stack.enter_context(tc.tile_pool(name="pjp", bufs=2, space="PSUM"))
            trp = stack.enter_context(tc.tile_pool(name="trp", bufs=2, space="PSUM"))
            hin_b, hin_f = h0_b, h0_f
            for layer in range(2):
                hout_b = hpool.tile([128, 16, TOK], BF16, tag="hb")
                if layer == 0:
                    hout_f = hpool.tile([128, 16, TOK], F8, tag="hf")
                else:
                    hout_f = None
                for j in range(16):
                    wslab = wpool.tile([128, 2, 2, 8, 2, 128], F8, tag="whw")
                    nc.sync.dma_start(out=wslab[:], in_=whw8[layer, j].rearrange(
                        "p (hl half c s o) -> p hl half c s o",
                        hl=2, half=2, c=8, s=2))
                    Pj = hwp.tile([128, 2, TOK], FP32, tag="pshw", name="pshw")
                    for half in range(2):
                        pdst = Pj[:, half, :]
                        for hl in range(2):
                            for c in range(8):
                                nc.tensor.matmul(
                                    out=pdst,
                                    lhsT=wslab[:, hl, half, c],
                                    rhs=hin_f[:, 2 * c:2 * c + 2, :],
                                    start=(hl == 0 and c == 0),
                                    stop=(hl == 1 and c == 7),
                                    perf_mode=DR)
                    nl = spool.tile([128, TOK], BF16, tag="nl")
                    gt = spool.tile([128, TOK], BF16, tag="gt")
                    nc.scalar.activation(
                        out=nl[:], in_=Pj[:, 0, :],
                        func=mybir.ActivationFunctionType.Relu,
                        bias=bhw_t[:, layer, j, 0:1],
                        scale=schw_t[:, layer:layer + 1])
                    nc.scalar.activation(
                        out=gt[:], in_=Pj[:, 1, :],
                        func=mybir.ActivationFunctionType.Sigmoid,
                        bias=bhw_t[:, layer, j, 1:2],
                        scale=schw_t[:, layer:layer + 1])
                    d = spool.tile([128, TOK], BF16, tag="d")
                    nc.vector.tensor_tensor(
                        out=d[:], in0=hin_b[:, j, :], in1=nl[:],
                        op=mybir.AluOpType.subtract)
                    nc.vector.tensor_mul(out=d[:], in0=gt[:], in1=d[:])
                    nc.vector.tensor_add(out=hout_b[:, j, :], in0=nl[:], in1=d[:])
                    if hout_f is not None:
                        nc.vector.tensor_scalar_mul(
                            out=hout_f[:, j, :], in0=hout_b[:, j, :], scalar1=SH)
                hin_b, hin_f = hout_b, hout_f

            # ---- 6. projection (bf16) + transpose + out
            for j2 in range(4):
                wp = wppool.tile([128, 16, 128], BF16, tag="wp")
                nc.sync.dma_start(out=wp[:], in_=wproj[j2])
                Pp = pjp.tile([128, TOK], FP32, tag="pspj", name="psproj")
                p_o = Pp[:]
                for c in range(16):
                    nc.tensor.matmul(
                        out=p_o, lhsT=wp[:, c, :], rhs=hin_b[:, c, :],
                        start=(c == 0), stop=(c == 15))
                ot = spool.tile([128, TOK], FP32, tag="ot")
                nc.scalar.activation(
                    out=ot[:], in_=p_o,
                    func=mybir.ActivationFunctionType.Identity,
                    bias=bproj_t[:, j2:j2 + 1], scale=1.0)
                for m4 in range(4):
                    p_t = trp.tile([128, TOK], FP32, tag="pstr", name="pstr")[:, 0:128]
                    nc.tensor.transpose(
                        out=p_t[:], in_=ot[:, 128 * m4:128 * (m4 + 1)],
                        identity=ident_t[:])
                    ob = spool.tile([128, 128], FP32, tag="ob")
                    nc.scalar.copy(out=ob[:], in_=p_t[:])
                    nc.sync.dma_start(
                        out=out[128 * m4:128 * (m4 + 1), 128 * j2:128 * (j2 + 1)],
                        in_=ob[:])
            stack.close()

    nc.compile()
    return nc


_CACHED = {}


def _pow2scale(am, target):
    return 2.0 ** np.floor(np.log2(target / max(am, 1e-20)))


def _prep(inputs):
    """Host-side layout prep: sharding, index arithmetic, weight packing."""
    chars = np.asarray(inputs["chars"]).astype(np.int64).reshape(NTOK, L)

    emb = np.asarray(inputs["char_emb"], np.float32)
    table = np.zeros((PAD_V, 128), np.float32)
    table[:CHAR_VOCAB, :CHAR_DIM] = emb
    table = table.astype(ml_dtypes.bfloat16)

    # conv weights -> (112, 2048) zero-padded taps, matching X_stack rows 16k+d
    wc = np.zeros((7, CHAR_DIM, N_FILTERS), np.float32)
    off = 0
    for fi, (w, n) in enumerate(FILTERS):
        cw = np.asarray(inputs[f"conv_w_{fi}"], np.float32)  # (n, 16, w)
        wc[:w, :, off:off + n] = cw.transpose(2, 1, 0)
        off += n
    wconv = wc.reshape(KDIM, N_FILTERS).astype(ml_dtypes.bfloat16)
    bconv = np.concatenate([np.asarray(inputs[f"conv_b_{i}"], np.float32)
                            for i in range(7)])
    bconv_dev = bconv.reshape(16, 128).T.copy()  # (128, 16)

    # highway weights: fp8 split, DoubleRow layout
    # whw8[l, j, p, (hl, half, c, s2, o)], value = Wsplit[hl][ic, col]
    # with ic = 256c + 128*s2 + p, col = 2048*half + 128j + o
    whw8 = np.zeros((2, 16, 128, 8192), ml_dtypes.float8_e4m3)
    bhw = np.zeros((128, 2, 16, 2), np.float32)
    schw = np.zeros((128, 2), np.float32)
    for l in range(2):
        W = np.asarray(inputs[f"hw_w_{l}"], np.float32)   # (4096, 2048)
        bb = np.asarray(inputs[f"hw_b_{l}"], np.float32)  # (4096,)
        WT = W.T  # (2048, 4096)
        s = _pow2scale(np.abs(WT).max(), 120.0)
        Whi = (WT * s).astype(ml_dtypes.float8_e4m3)
        Wlo = (WT * s - Whi.astype(np.float32)).astype(ml_dtypes.float8_e4m3)
        A = np.stack([Whi, Wlo]).astype(np.float32)       # (hl, 2048, 4096)
        A = A.reshape(2, 8, 2, 128, 2, 16, 128)           # hl c s2 p half j o
        A = A.transpose(5, 3, 0, 4, 1, 2, 6)              # j p hl half c s2 o
        whw8[l] = A.reshape(16, 128, 8192).astype(ml_dtypes.float8_e4m3)
        schw[:, l] = 1.0 / (s * SH)
        for j in range(16):
            bhw[:, l, j, 0] = bb[128 * j:128 * (j + 1)]
            bhw[:, l, j, 1] = bb[2048 + 128 * j:2048 + 128 * (j + 1)]

    Wp = np.asarray(inputs["proj_w"], np.float32)  # (512, 2048)
    WpT = Wp.T  # (2048, 512)
    # wproj[j2, p, c, o] = WpT[128c + p, 128j2 + o]
    wproj = WpT.reshape(16, 128, 4, 128).transpose(2, 1, 0, 3).astype(
        ml_dtypes.bfloat16).copy()
    bproj = np.zeros((128, 4), np.float32)
    bp = np.asarray(inputs["proj_b"], np.float32)
    for j2 in range(4):
        bproj[:, j2] = bp[128 * j2:128 * (j2 + 1)]

    ident = np.eye(128, dtype=np.float32)

    shared = dict(table=table, wconv=wconv, bconv=bconv_dev, whw8=whw8,
                  bhw=bhw, schw=schw, wproj=wproj, bproj=bproj, ident=ident)

    in_maps = []
    for core in range(N_CORES):
        cp = chars[core * TOK:(core + 1) * TOK]  # (512, 50)
        # flat index j = t'*512 + n  ->  idx_flat[j] = cp[n, t']
        idx_flat = cp.T.reshape(-1).astype(np.int16)  # (25600,)
        idx16 = idx_flat.reshape(NI // 16, 16).T.copy()  # (16, S)
        idx16 = np.tile(idx16, (8, 1))  # (128, S)
        m = dict(shared)
        m["idx"] = idx16
        in_maps.append(m)
    return in_maps


def kernel(**inputs) -> np.ndarray:
    if "nc" not in _CACHED:
        _CACHED["nc"] = build_module()
    nc = _CACHED["nc"]
    in_maps = _prep(inputs)
    res = run_bass_kernel_spmd(nc, in_maps, core_ids=list(range(N_CORES)))
    full = np.concatenate([r["out"] for r in res.results], axis=0)
    return full.reshape(B, S, PROJ_DIM)


# revision 68
# speedup vs baseline: 1.0151x; 1.0151x over previous
"""CharCNN token embedder (ELMo-style) on 8 Trainium2 NeuronCores.

Data-parallel over the 4096 = 16*256 tokens (512 per core). Weights replicated.

Per-core pipeline:
  1. dma_gather (transpose mode) pulls char-embedding rows (padded to 256B)
     into feature-major layout X[d, (t', n)] for t' in [0,56), n in [0,512).
  2. 7 shifted SBUF->SBUF copies build the patch matrix Xs[(k,d), (t, n)]
     (112 x 25600) = im2col for a width-7 window (weights zero-padded).
  3. Conv = bf16 matmuls with K=112: per 128-channel tile, one matmul per
     valid position t (N=512 tokens), PSUM rounds of 4 banks.
  4. Max-pool over positions per 4-bank PSUM round: ACT copies banks 1,3
     to SBUF, DVE pair-maxes them against banks 0,2 (one PSUM operand per
     DVE op -- a walrus requirement), then folds into two alternating
     accumulator chains; relu+bias at tile finalize; h written both bf16
     and fp8 (x64 scale).
  5. 2 highway layers in fp8e4 DoubleRow split precision: W ~ (Whi+Wlo)/s,
     K=256 per DR matmul, 16 DR matmuls per (j, half) accumulate one PSUM
     bank; ACT applies 1/(s*64) scale (per-partition AP) + bias + relu /
     sigmoid; DVE does the highway gating in bf16 4x mode.
  6. Projection to 512 in bf16, bias, PE-transpose to token-major, DMA out.
"""

from contextlib import ExitStack

import numpy as np
import ml_dtypes

import concourse.bass as bass
import concourse.mybir as mybir
import concourse.tile as tile
from concourse import bacc
from concourse.bass_utils import run_bass_kernel_spmd
from concourse.vector_clock import ScopedClock

# ---------------------------------------------------------------- constants
B, S, L = 16, 256, 50
CHAR_DIM = 16
CHAR_VOCAB = 262
PAD_V = 264            # table rows (262 real + 1 zero pad row + 1 spare)
ZERO_ROW = 262
FILTERS = [(1, 32), (2, 32), (3, 64), (4, 128), (5, 256), (6, 512), (7, 1024)]
N_FILTERS = 2048
PROJ_DIM = 512
N_CORES = 8
NTOK = B * S                  # 4096
TOK = NTOK // N_CORES         # 512 tokens per core
NPOS = 50                     # conv output positions computed
NI = TOK * NPOS               # gather indices per core = 25600
GATHER_CHUNK = 6400           # indices per dma_gather (descriptor-ring safe)
FREE = TOK * NPOS             # X_stack free size = 25600
KDIM = 112                    # 7 taps * 16 dims
SH = 64.0                     # fp8 activation scale for h0/h1 (absmax ~0.65)

# per 128-channel tile: full-row position count + (t, row_hi) row-tails
CH_TILES = [{"t_main": 48, "tails": [(48, 64), (49, 32)]}]
CH_TILES.append({"t_main": 47, "tails": []})      # w4
for _ in range(2):
    CH_TILES.append({"t_main": 46, "tails": []})  # w5
for _ in range(4):
    CH_TILES.append({"t_main": 45, "tails": []})  # w6
for _ in range(8):
    CH_TILES.append({"t_main": 44, "tails": []})  # w7
R_POS = 4

BF16 = mybir.dt.bfloat16
FP32 = mybir.dt.float32
F8 = mybir.dt.float8e4
MAX_OP = mybir.AluOpType.max

_MAX_WAITS_PER_INST = 1


def _patched_drain_and_barrier(self, tick_clock, wait_clock):
    # The walrus build in this container rejects CTRL instructions carrying
    # more than one sem wait; spread the kernel-tail drain waits over NOPs.
    nc = self.nc
    carrier = nc.sync.nop()
    wait_clock.add_sem_waits(carrier.ins, ScopedClock({None: tick_clock.global_clock}))
    si = carrier.ins.sync_info
    waits = list(si.on_wait) if si is not None and si.on_wait else []
    if len(waits) > _MAX_WAITS_PER_INST:
        carrier.ins.sync_info = mybir.SyncInfo(
            on_wait=waits[:_MAX_WAITS_PER_INST],
            on_update=list(si.on_update) if si.on_update else [])
        for i in range(_MAX_WAITS_PER_INST, len(waits), _MAX_WAITS_PER_INST):
            extra = nc.sync.nop()
            extra.ins.sync_info = mybir.SyncInfo(
                on_wait=waits[i:i + _MAX_WAITS_PER_INST], on_update=[])
    nc.sync.drain()
    nc.all_engine_barrier()
    assert self.sems is not None
    popped = nc._tile_sem_poison_stack.pop()
    assert popped is self._sem_poison
    nc.clear_and_free_semaphores(list(self.sems.allocated().values()))
    nc.all_engine_barrier()


tile.TileContext._drain_and_barrier = _patched_drain_and_barrier


class PoolSched:
    """Greedy engine-load balancer for the conv max-pool stage.

    Cost constants are exact TimelineSim per-instruction engine times."""

    def __init__(self, nc, spool):
        self.nc = nc
        self.spool = spool
        self.est = {"dve": 0.0, "act": 0.0, "pool": 0.0}

    def _pick(self, options):
        """options: list of (key, {eng: cost}). Pick min resulting max-load."""
        best, bestv = None, None
        for key, costs in options:
            peak = max(self.est[e] + costs.get(e, 0.0) for e in self.est)
            if bestv is None or peak < bestv:
                best, bestv = (key, costs), peak
        for e, c in best[1].items():
            self.est[e] += c
        return best[0]

    def fold(self, acc_ap, in_ap, nelem):
        """acc = max(acc, in); DVE only (walrus rejects TT on gpsimd)."""
        self.est["dve"] += {512: 328.0, 1024: 594.0, 2048: 1127.0}[nelem]
        self.nc.vector.tensor_tensor(
            out=acc_ap, in0=acc_ap, in1=in_ap, op=MAX_OP)

    def round(self, P, nt, st, ridx):
        """Drain one PSUM round (nt banks) into an independent chain.

        walrus allows at most ONE PSUM operand per DVE op, so drains are
        either a fused TT(PSUM, acc_sbuf)->acc (quad chains D1/D2 on DVE)
        or an ACT copy + SBUF-side fold (quad chain C).  acc16 slots:
        0:4 = D1, 4:8 = D2, 8:12 = C."""
        nc, spool = self.nc, self.spool
        acc = st["acc16"]
        if nt == 4:
            path = self._pick([("dve", {"dve": 2258.0}),
                               ("act", {"act": 1992.0, "dve": 0.0})])
            if path == "dve":
                k = st["dsel"]
                st["dsel"] ^= 1
                sl = 4 * k
                if st["first"][k]:
                    nc.vector.tensor_copy(out=acc[:, sl:sl + 4, :], in_=P[:, 0:4, :])
                    st["first"][k] = False
                else:
                    nc.vector.tensor_tensor(
                        out=acc[:, sl:sl + 4, :], in0=P[:, 0:4, :],
                        in1=acc[:, sl:sl + 4, :], op=MAX_OP)
            else:
                k = st["csel"]
                st["csel"] ^= 1
                sl = 8 + 4 * k
                if st["first"][2 + k]:
                    nc.scalar.copy(out=acc[:, sl:sl + 4, :], in_=P[:, 0:4, :])
                    st["first"][2 + k] = False
                else:
                    t = spool.tile([128, 4, TOK], BF16, tag="t4", name="tdr4")
                    nc.scalar.copy(out=t[:], in_=P[:, 0:4, :])
                    self.fold(acc[:, sl:sl + 4, :], t[:], 2048)
        else:
            # tail rounds (1-3 banks): fused max into chain D1's prefix
            if st["first"][0]:
                nc.vector.tensor_copy(out=acc[:, 0:nt, :], in_=P[:, 0:nt, :])
                st["first"][0] = False
                # remaining D1 slots stay virgin: seed them too
                if nt < 4:
                    nc.vector.tensor_copy(out=acc[:, nt:4, :],
                                          in_=P[:, 0:4 - nt, :])
            else:
                nc.vector.tensor_tensor(
                    out=acc[:, 0:nt, :], in0=P[:, 0:nt, :],
                    in1=acc[:, 0:nt, :], op=MAX_OP)
            self.est["dve"] += {1: 658.0, 2: 1192.0, 3: 1725.0}[nt]

    def finalize(self, st, h_b, h_f, bias_ap, i):
        nc, spool = self.nc, self.spool
        acc = st["acc16"]
        # combine quad chains (DVE)
        srcs = [acc[:, 0:4, :]]
        for k in (1, 2, 3):
            if not st["first"][k]:
                srcs.append(acc[:, 4 * k:4 * k + 4, :])
        while len(srcs) > 1:
            b = srcs.pop()
            a = srcs[-1]
            self.est["dve"] += 1127.0
            nc.vector.tensor_tensor(out=a, in0=a, in1=b, op=MAX_OP)
        m2 = spool.tile([128, 2, TOK], BF16, tag="t2")
        nc.vector.tensor_tensor(
            out=m2[:], in0=acc[:, 0:2, :], in1=acc[:, 2:4, :], op=MAX_OP)
        nc.vector.tensor_tensor(
            out=m2[:, 0:1, :], in0=m2[:, 0:1, :], in1=m2[:, 1:2, :], op=MAX_OP)
        self.est["dve"] += 594.0 + 328.0
        nc.scalar.activation(
            out=h_b[:, i, :], in_=m2[:, 0, :],
            func=mybir.ActivationFunctionType.Relu,
            bias=bias_ap, scale=1.0)
        self.est["act"] += 712.0
        if h_f is not None:
            self.est["dve"] += 327.0
            nc.vector.tensor_scalar_mul(
                out=h_f[:, i, :], in0=h_b[:, i, :], scalar1=SH)


# ---------------------------------------------------------------- device IR
def build_module():
    nc = bacc.Bacc()
    SIdx = NI // 16  # 1792 int16 columns

    table = nc.dram_tensor("table", [PAD_V, 128], BF16, kind="ExternalInput")
    idx = nc.dram_tensor("idx", [128, SIdx], mybir.dt.int16, kind="ExternalInput")
    wconv = nc.dram_tensor("wconv", [KDIM, N_FILTERS], BF16, kind="ExternalInput")
    bconv = nc.dram_tensor("bconv", [128, 16], FP32, kind="ExternalInput")
    # fp8 split highway weights: [l, j, p, (hl, half, c, s2, o)] flattened
    whw8 = nc.dram_tensor("whw8", [2, 16, 128, 8192], F8, kind="ExternalInput")
    bhw = nc.dram_tensor("bhw", [128, 2, 16, 2], FP32, kind="ExternalInput")
    schw = nc.dram_tensor("schw", [128, 2], FP32, kind="ExternalInput")
    wproj = nc.dram_tensor("wproj", [4, 128, 16, 128], BF16, kind="ExternalInput")
    bproj = nc.dram_tensor("bproj", [128, 4], FP32, kind="ExternalInput")
    ident = nc.dram_tensor("ident", [128, 128], FP32, kind="ExternalInput")
    out = nc.dram_tensor("out", [TOK, PROJ_DIM], FP32, kind="ExternalOutput")

    with tile.TileContext(nc) as tc:
        with (
            tc.tile_pool(name="consts", bufs=1) as cpool,
            tc.tile_pool(name="hbuf", bufs=2) as hpool,
            tc.tile_pool(name="wstream", bufs=3) as wpool,
            tc.tile_pool(name="wproj", bufs=2) as wppool,
            tc.tile_pool(name="small", bufs=2) as spool,
        ):
            # xs lives only through the conv phase; its scoped pool frees
            # 50KB for the highway weight stream
            with tc.tile_pool(name="xsp", bufs=1) as xspool:
                # ---- 1+2. gather char embeddings chunk-by-chunk into a
                # small ring buffer; scatter each chunk into the 7 tap bands
                # of the patch matrix as it lands (pipelines gather vs copy).
                with tc.tile_pool(name="gather", bufs=2) as gpool:
                    # idx first on the DMA queue: it gates the gather DGE
                    idx_t = gpool.tile([128, SIdx], mybir.dt.int16, tag="idx")
                    nc.sync.dma_start(out=idx_t[:], in_=idx[:])
                    wconv_t = cpool.tile([KDIM, N_FILTERS], BF16)
                    nc.sync.dma_start(out=wconv_t[:], in_=wconv[:])
                    bconv_t = cpool.tile([128, 16], FP32)
                    nc.sync.dma_start(out=bconv_t[:], in_=bconv[:])
                    bhw_t = cpool.tile([128, 2, 16, 2], FP32)
                    nc.sync.dma_start(out=bhw_t[:], in_=bhw[:])
                    schw_t = cpool.tile([128, 2], FP32)
                    nc.sync.dma_start(out=schw_t[:], in_=schw[:])
                    bproj_t = cpool.tile([128, 4], FP32)
                    nc.sync.dma_start(out=bproj_t[:], in_=bproj[:])
                    ident_t = cpool.tile([128, 128], FP32)
                    nc.sync.dma_start(out=ident_t[:], in_=ident[:])

                    xs = xspool.tile([KDIM, FREE], BF16)
                    # tap band k's last 512k cols correspond to char
                    # positions >= 50 and must read as zero.  One
                    # partition-0-based memset over the union region; the
                    # tap copies below rewrite the valid parts.
                    nc.vector.memset(xs[:, FREE - 512 * 6:], 0)

                    for r in range(NI // GATHER_CHUNK):
                        o = r * GATHER_CHUNK
                        xgc = gpool.tile([128, 1, GATHER_CHUNK], BF16, tag="xgc")
                        nc.gpsimd.dma_gather(
                            out_ap=xgc[:],
                            in_ap=table[:],
                            idxs_ap=idx_t[:, o // 16:(o + GATHER_CHUNK) // 16],
                            num_idxs=GATHER_CHUNK,
                            num_idxs_reg=GATHER_CHUNK,
                            elem_size=128,
                            transpose=True,
                            single_packet=False,
                        )
                        for k in range(7):
                            # xs[16k+d, c] = xg[d, c + 512k]; chunk covers
                            # xg cols [o, o+CHUNK)
                            lo = max(0, o - 512 * k)
                            hi = min(FREE - 512 * k, o + GATHER_CHUNK - 512 * k)
                            if lo >= hi:
                                continue
                            nc.sync.dma_start(
                                out=xs[16 * k:16 * (k + 1), lo:hi],
                                in_=xgc[0:16, 0, lo + 512 * k - o:hi + 512 * k - o],
                            )

                # ---- 3+4. conv + max pool + relu -> h tiles (bf16 + fp8)
                # 2-bank PSUM rounds in a 4-deep ring: drain latency hides
                # behind 3 rounds of PE lookahead.  Per round, greedy pick:
                #  - A2: one 2-bank ACT copy + DVE pair-fold   (ACT 1138/DVE 594)
                #  - D2: ACT copies bank1, DVE maxes bank0 vs it, slab-fold
                #                                              (ACT 712/DVE 986)
                # Four slab chains in acc4 keep folds off the critical path.
                with tc.tile_pool(name="convp", bufs=4, space="PSUM") as convp:
                    h0_b = hpool.tile([128, 16, TOK], BF16, tag="hb")
                    h0_f = hpool.tile([128, 16, TOK], F8, tag="hf")
                    est = {"dve": 0.0, "act": 0.0}
                    for i, spec in enumerate(CH_TILES):
                        lhsT = wconv_t[:, 128 * i:128 * (i + 1)]
                        acc4 = spool.tile([128, 4, TOK], BF16, tag="acc4")
                        first4 = [True, True, True, True]
                        t_cnt = spec["t_main"]
                        ridx = 0
                        t0 = 0
                        while t0 < t_cnt:
                            nt = min(2, t_cnt - t0)
                            P = convp.tile([128, 2, TOK], FP32, tag="ps")
                            for r in range(nt):
                                t = t0 + r
                                nc.tensor.matmul(
                                    out=P[:, r, :], lhsT=lhsT,
                                    rhs=xs[:, TOK * t:TOK * (t + 1)],
                                    start=True, stop=True)
                            if nt == 2:
                                pair = ridx % 2
                                sl = 2 * pair
                                slot = ridx % 4
                                a_first = first4[sl] and first4[sl + 1]
                                # strict D,A,D cycle (f_d=2/3 balances DVE/ACT)
                                # after two forced A2 seed rounds; determinism
                                # avoids greedy burstiness -> steadier pipeline
                                use_a2 = (ridx < 2) or (ridx % 3 == 1)
                                if use_a2:
                                    # A2: 2-bank ACT copy (forced for the
                                    # first two rounds to seed all 4 slots)
                                    est["act"] += 1138.0
                                    if a_first:
                                        nc.scalar.copy(out=acc4[:, sl:sl + 2, :],
                                                       in_=P[:, 0:2, :])
                                        first4[sl] = first4[sl + 1] = False
                                    else:
                                        t2 = spool.tile([128, 2, TOK], BF16,
                                                        tag="sp", name="t2a")
                                        nc.scalar.copy(out=t2[:], in_=P[:, 0:2, :])
                                        nc.vector.tensor_tensor(
                                            out=acc4[:, sl:sl + 2, :],
                                            in0=acc4[:, sl:sl + 2, :],
                                            in1=t2[:], op=MAX_OP)
                                        est["dve"] += 594.0
                                else:
                                    # D2: bank1 via ACT, bank0 via DVE max
                                    c = spool.tile([128, TOK], BF16, tag="c0",
                                                   name="cd")
                                    nc.scalar.copy(out=c[:], in_=P[:, 1, :])
                                    est["act"] += 712.0
                                    if first4[slot]:
                                        nc.vector.tensor_tensor(
                                            out=acc4[:, slot:slot + 1, :],
                                            in0=P[:, 0:1, :],
                                            in1=c[:].unsqueeze(1), op=MAX_OP)
                                        first4[slot] = False
                                        est["dve"] += 658.0
                                    else:
                                        tl = spool.tile([128, 2, TOK], BF16,
                                                        tag="sp", name="tld")
                                        nc.vector.tensor_tensor(
                                            out=tl[:, 0:1, :], in0=P[:, 0:1, :],
                                            in1=c[:].unsqueeze(1), op=MAX_OP)
                                        nc.vector.tensor_tensor(
                                            out=acc4[:, slot:slot + 1, :],
                                            in0=acc4[:, slot:slot + 1, :],
                                            in1=tl[:, 0:1, :], op=MAX_OP)
                                        est["dve"] += 986.0
                            else:
                                # single-bank tail (odd t_cnt)
                                slot = ridx % 4
                                tl = spool.tile([128, 2, TOK], BF16, tag="sp",
                                                name="tl1")
                                nc.scalar.copy(out=tl[:, 0:1, :], in_=P[:, 0:1, :])
                                est["act"] += 712.0
                                nc.vector.tensor_tensor(
                                    out=acc4[:, slot:slot + 1, :],
                                    in0=acc4[:, slot:slot + 1, :],
                                    in1=tl[:, 0:1, :], op=MAX_OP)
                                est["dve"] += 328.0
                            ridx += 1
                            t0 += nt
                        # row-tails (tile 0): positions valid for a row subset
                        if spec["tails"]:
                            P = convp.tile([128, 2, TOK], FP32, tag="ps")
                            for r, (t, hi) in enumerate(spec["tails"]):
                                nc.tensor.matmul(
                                    out=P[:, r, :], lhsT=lhsT,
                                    rhs=xs[:, TOK * t:TOK * (t + 1)],
                                    start=True, stop=True)
                            for r, (t, hi) in enumerate(spec["tails"]):
                                tl = spool.tile([128, 2, TOK], BF16, tag="sp",
                                                name="tlr")
                                nc.scalar.copy(out=tl[:, 0:1, :],
                                               in_=P[:, r:r + 1, :])
                                nc.vector.tensor_tensor(
                                    out=acc4[0:hi, 0:1, :],
                                    in0=acc4[0:hi, 0:1, :],
                                    in1=tl[0:hi, 0:1, :], op=MAX_OP)
                                est["act"] += 712.0
                                est["dve"] += 328.0
                        m2 = spool.tile([128, 2, TOK], BF16, tag="m2")
                        nc.vector.tensor_tensor(
                            out=m2[:], in0=acc4[:, 0:2, :], in1=acc4[:, 2:4, :],
                            op=MAX_OP)
                        v = spool.tile([128, TOK], BF16, tag="vv")
                        nc.vector.tensor_tensor(
                            out=v[:], in0=m2[:, 0, :], in1=m2[:, 1, :],
                            op=MAX_OP)
                        est["dve"] += 594.0 + 328.0
                        nc.scalar.activation(
                            out=h0_b[:, i, :], in_=v[:],
                            func=mybir.ActivationFunctionType.Relu,
                            bias=bconv_t[:, i:i + 1], scale=1.0)
                        est["act"] += 712.0
                        nc.vector.tensor_scalar_mul(
                            out=h0_f[:, i, :], in0=h0_b[:, i, :], scalar1=SH)
                        est["dve"] += 327.0

            # ---- 5. highway layers: fp8 DoubleRow split precision
            DR = mybir.MatmulPerfMode.DoubleRow
            stack = ExitStack()
            hwp = stack.enter_context(tc.tile_pool(name="hwp", bufs=2, space="PSUM"))
            pjp = stack.enter_context(tc.tile_pool(name="pjp", bufs=2, space="PSUM"))
            trp = stack.enter_context(tc.tile_pool(name="trp", bufs=2, space="PSUM"))
            hin_b, hin_f = h0_b, h0_f
            for layer in range(2):
                hout_b = hpool.tile([128, 16, TOK], BF16, tag="hb")
                if layer == 0:
                    hout_f = hpool.tile([128, 16, TOK], F8, tag="hf")
                else:
                    hout_f = None
                for j in range(16):
                    wslab = wpool.tile([128, 2, 2, 8, 2, 128], F8, tag="whw")
                    nc.sync.dma_start(out=wslab[:], in_=whw8[layer, j].rearrange(
                        "p (hl half c s o) -> p hl half c s o",
                        hl=2, half=2, c=8, s=2))
                    Pj = hwp.tile([128, 2, TOK], FP32, tag="pshw", name="pshw")
                    for half in range(2):
                        pdst = Pj[:, half, :]
                        # layer-1 gate half: hi chain only -- sigmoid squashes
                        # the ~6% lo-term and its error does not compound
                        # through another highway layer
                        n_hl = 1 if (layer == 1 and half == 1) else 2
                        for hl in range(n_hl):
                            for c in range(8):
                                nc.tensor.matmul(
                                    out=pdst,
                                    lhsT=wslab[:, hl, half, c],
                                    rhs=hin_f[:, 2 * c:2 * c + 2, :],
                                    start=(hl == 0 and c == 0),
                                    stop=(hl == n_hl - 1 and c == 7),
                                    perf_mode=DR)
                    nl = spool.tile([128, TOK], BF16, tag="nl")
                    gt = spool.tile([128, TOK], BF16, tag="gt")
                    nc.scalar.activation(
                        out=nl[:], in_=Pj[:, 0, :],
                        func=mybir.ActivationFunctionType.Relu,
                        bias=bhw_t[:, layer, j, 0:1],
                        scale=schw_t[:, layer:layer + 1])
                    nc.scalar.activation(
                        out=gt[:], in_=Pj[:, 1, :],
                        func=mybir.ActivationFunctionType.Sigmoid,
                        bias=bhw_t[:, layer, j, 1:2],
                        scale=schw_t[:, layer:layer + 1])
                    d = spool.tile([128, TOK], BF16, tag="d")
                    nc.vector.tensor_tensor(
                        out=d[:], in0=hin_b[:, j, :], in1=nl[:],
                        op=mybir.AluOpType.subtract)
                    nc.vector.tensor_mul(out=d[:], in0=gt[:], in1=d[:])
                    nc.vector.tensor_add(out=hout_b[:, j, :], in0=nl[:], in1=d[:])
                    if hout_f is not None:
                        nc.vector.tensor_scalar_mul(
                            out=hout_f[:, j, :], in0=hout_b[:, j, :], scalar1=SH)
                hin_b, hin_f = hout_b, hout_f

            # ---- 6. projection (bf16) + transpose + out
            for j2 in range(4):
                wp = wppool.tile([128, 16, 128], BF16, tag="wp")
                nc.sync.dma_start(out=wp[:], in_=wproj[j2])
                Pp = pjp.tile([128, TOK], FP32, tag="pspj", name="psproj")
                p_o = Pp[:]
                for c in range(16):
                    nc.tensor.matmul(
                        out=p_o, lhsT=wp[:, c, :], rhs=hin_b[:, c, :],
                        start=(c == 0), stop=(c == 15))
                ot = spool.tile([128, TOK], FP32, tag="ot")
                nc.scalar.activation(
                    out=ot[:], in_=p_o,
                    func=mybir.ActivationFunctionType.Identity,
                    bias=bproj_t[:, j2:j2 + 1], scale=1.0)
                for m4 in range(4):
                    p_t = trp.tile([128, TOK], FP32, tag="pstr", name="pstr")[:, 0:128]
                    nc.tensor.transpose(
                        out=p_t[:], in_=ot[:, 128 * m4:128 * (m4 + 1)],
                        identity=ident_t[:])
                    ob = spool.tile([128, 128], FP32, tag="ob")
                    nc.scalar.copy(out=ob[:], in_=p_t[:])
                    nc.sync.dma_start(
                        out=out[128 * m4:128 * (m4 + 1), 128 * j2:128 * (j2 + 1)],
                        in_=ob[:])
            stack.close()

    nc.compile()
    return nc


_CACHED = {}


def _pow2scale(am, target):
    return 2.0 ** np.floor(np.log2(target / max(am, 1e-20)))


def _prep(inputs):
    """Host-side layout prep: sharding, index arithmetic, weight packing."""
    chars = np.asarray(inputs["chars"]).astype(np.int64).reshape(NTOK, L)

    emb = np.asarray(inputs["char_emb"], np.float32)
    table = np.zeros((PAD_V, 128), np.float32)
    table[:CHAR_VOCAB, :CHAR_DIM] = emb
    table = table.astype(ml_dtypes.bfloat16)

    # conv weights -> (112, 2048) zero-padded taps, matching X_stack rows 16k+d
    wc = np.zeros((7, CHAR_DIM, N_FILTERS), np.float32)
    off = 0
    for fi, (w, n) in enumerate(FILTERS):
        cw = np.asarray(inputs[f"conv_w_{fi}"], np.float32)  # (n, 16, w)
        wc[:w, :, off:off + n] = cw.transpose(2, 1, 0)
        off += n
    wconv = wc.reshape(KDIM, N_FILTERS).astype(ml_dtypes.bfloat16)
    bconv = np.concatenate([np.asarray(inputs[f"conv_b_{i}"], np.float32)
                            for i in range(7)])
    bconv_dev = bconv.reshape(16, 128).T.copy()  # (128, 16)

    # highway weights: fp8 split, DoubleRow layout
    # whw8[l, j, p, (hl, half, c, s2, o)], value = Wsplit[hl][ic, col]
    # with ic = 256c + 128*s2 + p, col = 2048*half + 128j + o
    whw8 = np.zeros((2, 16, 128, 8192), ml_dtypes.float8_e4m3)
    bhw = np.zeros((128, 2, 16, 2), np.float32)
    schw = np.zeros((128, 2), np.float32)
    for l in range(2):
        W = np.asarray(inputs[f"hw_w_{l}"], np.float32)   # (4096, 2048)
        bb = np.asarray(inputs[f"hw_b_{l}"], np.float32)  # (4096,)
        WT = W.T  # (2048, 4096)
        s = _pow2scale(np.abs(WT).max(), 120.0)
        Whi = (WT * s).astype(ml_dtypes.float8_e4m3)
        Wlo = (WT * s - Whi.astype(np.float32)).astype(ml_dtypes.float8_e4m3)
        A = np.stack([Whi, Wlo]).astype(np.float32)       # (hl, 2048, 4096)
        A = A.reshape(2, 8, 2, 128, 2, 16, 128)           # hl c s2 p half j o
        A = A.transpose(5, 3, 0, 4, 1, 2, 6)              # j p hl half c s2 o
        whw8[l] = A.reshape(16, 128, 8192).astype(ml_dtypes.float8_e4m3)
        schw[:, l] = 1.0 / (s * SH)
        for j in range(16):
            bhw[:, l, j, 0] = bb[128 * j:128 * (j + 1)]
            bhw[:, l, j, 1] = bb[2048 + 128 * j:2048 + 128 * (j + 1)]

    Wp = np.asarray(inputs["proj_w"], np.float32)  # (512, 2048)
    WpT = Wp.T  # (2048, 512)
    # wproj[j2, p, c, o] = WpT[128c + p, 128j2 + o]
    wproj = WpT.reshape(16, 128, 4, 128).transpose(2, 1, 0, 3).astype(
        ml_dtypes.bfloat16).copy()
    bproj = np.zeros((128, 4), np.float32)
    bp = np.asarray(inputs["proj_b"], np.float32)
    for j2 in range(4):
        bproj[:, j2] = bp[128 * j2:128 * (j2 + 1)]

    ident = np.eye(128, dtype=np.float32)

    shared = dict(table=table, wconv=wconv, bconv=bconv_dev, whw8=whw8,
                  bhw=bhw, schw=schw, wproj=wproj, bproj=bproj, ident=ident)

    in_maps = []
    for core in range(N_CORES):
        cp = chars[core * TOK:(core + 1) * TOK]  # (512, 50)
        # flat index j = t'*512 + n  ->  idx_flat[j] = cp[n, t']
        idx_flat = cp.T.reshape(-1).astype(np.int16)  # (25600,)
        idx16 = idx_flat.reshape(NI // 16, 16).T.copy()  # (16, S)
        idx16 = np.tile(idx16, (8, 1))  # (128, S)
        m = dict(shared)
        m["idx"] = idx16
        in_maps.append(m)
    return in_maps


def kernel(**inputs) -> np.ndarray:
    if "nc" not in _CACHED:
        _CACHED["nc"] = build_module()
    nc = _CACHED["nc"]
    in_maps = _prep(inputs)
    res = run_bass_kernel_spmd(nc, in_maps, core_ids=list(range(N_CORES)))
    full = np.concatenate([r["out"] for r in res.results], axis=0)
    return full.reshape(B, S, PROJ_DIM)


# revision 69
# speedup vs baseline: 1.0311x; 1.0158x over previous
"""CharCNN token embedder (ELMo-style) on 8 Trainium2 NeuronCores.

Data-parallel over the 4096 = 16*256 tokens (512 per core). Weights replicated.

Per-core pipeline:
  1. dma_gather (transpose mode) pulls char-embedding rows (padded to 256B)
     into feature-major layout X[d, (t', n)] for t' in [0,56), n in [0,512).
  2. 7 shifted SBUF->SBUF copies build the patch matrix Xs[(k,d), (t, n)]
     (112 x 25600) = im2col for a width-7 window (weights zero-padded).
  3. Conv = bf16 matmuls with K=112: per 128-channel tile, one matmul per
     valid position t (N=512 tokens), PSUM rounds of 4 banks.
  4. Max-pool over positions per 4-bank PSUM round: ACT copies banks 1,3
     to SBUF, DVE pair-maxes them against banks 0,2 (one PSUM operand per
     DVE op -- a walrus requirement), then folds into two alternating
     accumulator chains; relu+bias at tile finalize; h written both bf16
     and fp8 (x64 scale).
  5. 2 highway layers in fp8e4 DoubleRow split precision: W ~ (Whi+Wlo)/s,
     K=256 per DR matmul, 16 DR matmuls per (j, half) accumulate one PSUM
     bank; ACT applies 1/(s*64) scale (per-partition AP) + bias + relu /
     sigmoid; DVE does the highway gating in bf16 4x mode.
  6. Projection to 512 in bf16, bias, PE-transpose to token-major, DMA out.
"""

from contextlib import ExitStack

import numpy as np
import ml_dtypes

import concourse.bass as bass
import concourse.mybir as mybir
import concourse.tile as tile
from concourse import bacc
from concourse.bass_utils import run_bass_kernel_spmd
from concourse.vector_clock import ScopedClock

# ---------------------------------------------------------------- constants
B, S, L = 16, 256, 50
CHAR_DIM = 16
CHAR_VOCAB = 262
PAD_V = 264            # table rows (262 real + 1 zero pad row + 1 spare)
ZERO_ROW = 262
FILTERS = [(1, 32), (2, 32), (3, 64), (4, 128), (5, 256), (6, 512), (7, 1024)]
N_FILTERS = 2048
PROJ_DIM = 512
N_CORES = 8
NTOK = B * S                  # 4096
TOK = NTOK // N_CORES         # 512 tokens per core
NPOS = 50                     # conv output positions computed
NI = TOK * NPOS               # gather indices per core = 25600
GATHER_CHUNK = 6400           # indices per dma_gather (descriptor-ring safe)
FREE = TOK * NPOS             # X_stack free size = 25600
KDIM = 112                    # 7 taps * 16 dims
SH = 64.0                     # fp8 activation scale for h0/h1 (absmax ~0.65)

# per 128-channel tile: full-row position count + (t, row_hi) row-tails
CH_TILES = [{"t_main": 48, "tails": [(48, 64), (49, 32)]}]
CH_TILES.append({"t_main": 47, "tails": []})      # w4
for _ in range(2):
    CH_TILES.append({"t_main": 46, "tails": []})  # w5
for _ in range(4):
    CH_TILES.append({"t_main": 45, "tails": []})  # w6
for _ in range(8):
    CH_TILES.append({"t_main": 44, "tails": []})  # w7
R_POS = 4

BF16 = mybir.dt.bfloat16
FP32 = mybir.dt.float32
F8 = mybir.dt.float8e4
MAX_OP = mybir.AluOpType.max

_MAX_WAITS_PER_INST = 1


def _patched_drain_and_barrier(self, tick_clock, wait_clock):
    # The walrus build in this container rejects CTRL instructions carrying
    # more than one sem wait; spread the kernel-tail drain waits over NOPs.
    nc = self.nc
    carrier = nc.sync.nop()
    wait_clock.add_sem_waits(carrier.ins, ScopedClock({None: tick_clock.global_clock}))
    si = carrier.ins.sync_info
    waits = list(si.on_wait) if si is not None and si.on_wait else []
    if len(waits) > _MAX_WAITS_PER_INST:
        carrier.ins.sync_info = mybir.SyncInfo(
            on_wait=waits[:_MAX_WAITS_PER_INST],
            on_update=list(si.on_update) if si.on_update else [])
        for i in range(_MAX_WAITS_PER_INST, len(waits), _MAX_WAITS_PER_INST):
            extra = nc.sync.nop()
            extra.ins.sync_info = mybir.SyncInfo(
                on_wait=waits[i:i + _MAX_WAITS_PER_INST], on_update=[])
    nc.sync.drain()
    nc.all_engine_barrier()
    assert self.sems is not None
    popped = nc._tile_sem_poison_stack.pop()
    assert popped is self._sem_poison
    nc.clear_and_free_semaphores(list(self.sems.allocated().values()))
    nc.all_engine_barrier()


tile.TileContext._drain_and_barrier = _patched_drain_and_barrier


class PoolSched:
    """Greedy engine-load balancer for the conv max-pool stage.

    Cost constants are exact TimelineSim per-instruction engine times."""

    def __init__(self, nc, spool):
        self.nc = nc
        self.spool = spool
        self.est = {"dve": 0.0, "act": 0.0, "pool": 0.0}

    def _pick(self, options):
        """options: list of (key, {eng: cost}). Pick min resulting max-load."""
        best, bestv = None, None
        for key, costs in options:
            peak = max(self.est[e] + costs.get(e, 0.0) for e in self.est)
            if bestv is None or peak < bestv:
                best, bestv = (key, costs), peak
        for e, c in best[1].items():
            self.est[e] += c
        return best[0]

    def fold(self, acc_ap, in_ap, nelem):
        """acc = max(acc, in); DVE only (walrus rejects TT on gpsimd)."""
        self.est["dve"] += {512: 328.0, 1024: 594.0, 2048: 1127.0}[nelem]
        self.nc.vector.tensor_tensor(
            out=acc_ap, in0=acc_ap, in1=in_ap, op=MAX_OP)

    def round(self, P, nt, st, ridx):
        """Drain one PSUM round (nt banks) into an independent chain.

        walrus allows at most ONE PSUM operand per DVE op, so drains are
        either a fused TT(PSUM, acc_sbuf)->acc (quad chains D1/D2 on DVE)
        or an ACT copy + SBUF-side fold (quad chain C).  acc16 slots:
        0:4 = D1, 4:8 = D2, 8:12 = C."""
        nc, spool = self.nc, self.spool
        acc = st["acc16"]
        if nt == 4:
            path = self._pick([("dve", {"dve": 2258.0}),
                               ("act", {"act": 1992.0, "dve": 0.0})])
            if path == "dve":
                k = st["dsel"]
                st["dsel"] ^= 1
                sl = 4 * k
                if st["first"][k]:
                    nc.vector.tensor_copy(out=acc[:, sl:sl + 4, :], in_=P[:, 0:4, :])
                    st["first"][k] = False
                else:
                    nc.vector.tensor_tensor(
                        out=acc[:, sl:sl + 4, :], in0=P[:, 0:4, :],
                        in1=acc[:, sl:sl + 4, :], op=MAX_OP)
            else:
                k = st["csel"]
                st["csel"] ^= 1
                sl = 8 + 4 * k
                if st["first"][2 + k]:
                    nc.scalar.copy(out=acc[:, sl:sl + 4, :], in_=P[:, 0:4, :])
                    st["first"][2 + k] = False
                else:
                    t = spool.tile([128, 4, TOK], BF16, tag="t4", name="tdr4")
                    nc.scalar.copy(out=t[:], in_=P[:, 0:4, :])
                    self.fold(acc[:, sl:sl + 4, :], t[:], 2048)
        else:
            # tail rounds (1-3 banks): fused max into chain D1's prefix
            if st["first"][0]:
                nc.vector.tensor_copy(out=acc[:, 0:nt, :], in_=P[:, 0:nt, :])
                st["first"][0] = False
                # remaining D1 slots stay virgin: seed them too
                if nt < 4:
                    nc.vector.tensor_copy(out=acc[:, nt:4, :],
                                          in_=P[:, 0:4 - nt, :])
            else:
                nc.vector.tensor_tensor(
                    out=acc[:, 0:nt, :], in0=P[:, 0:nt, :],
                    in1=acc[:, 0:nt, :], op=MAX_OP)
            self.est["dve"] += {1: 658.0, 2: 1192.0, 3: 1725.0}[nt]

    def finalize(self, st, h_b, h_f, bias_ap, i):
        nc, spool = self.nc, self.spool
        acc = st["acc16"]
        # combine quad chains (DVE)
        srcs = [acc[:, 0:4, :]]
        for k in (1, 2, 3):
            if not st["first"][k]:
                srcs.append(acc[:, 4 * k:4 * k + 4, :])
        while len(srcs) > 1:
            b = srcs.pop()
            a = srcs[-1]
            self.est["dve"] += 1127.0
            nc.vector.tensor_tensor(out=a, in0=a, in1=b, op=MAX_OP)
        m2 = spool.tile([128, 2, TOK], BF16, tag="t2")
        nc.vector.tensor_tensor(
            out=m2[:], in0=acc[:, 0:2, :], in1=acc[:, 2:4, :], op=MAX_OP)
        nc.vector.tensor_tensor(
            out=m2[:, 0:1, :], in0=m2[:, 0:1, :], in1=m2[:, 1:2, :], op=MAX_OP)
        self.est["dve"] += 594.0 + 328.0
        nc.scalar.activation(
            out=h_b[:, i, :], in_=m2[:, 0, :],
            func=mybir.ActivationFunctionType.Relu,
            bias=bias_ap, scale=1.0)
        self.est["act"] += 712.0
        if h_f is not None:
            self.est["dve"] += 327.0
            nc.vector.tensor_scalar_mul(
                out=h_f[:, i, :], in0=h_b[:, i, :], scalar1=SH)


# ---------------------------------------------------------------- device IR
def build_module():
    nc = bacc.Bacc()
    SIdx = NI // 16  # 1792 int16 columns

    table = nc.dram_tensor("table", [PAD_V, 128], BF16, kind="ExternalInput")
    idx = nc.dram_tensor("idx", [128, SIdx], mybir.dt.int16, kind="ExternalInput")
    wconv = nc.dram_tensor("wconv", [KDIM, N_FILTERS], BF16, kind="ExternalInput")
    bconv = nc.dram_tensor("bconv", [128, 16], FP32, kind="ExternalInput")
    # fp8 split highway weights: [l, j, p, (hl, half, c, s2, o)] flattened
    whw8 = nc.dram_tensor("whw8", [2, 16, 128, 8192], F8, kind="ExternalInput")
    bhw = nc.dram_tensor("bhw", [128, 2, 16, 2], FP32, kind="ExternalInput")
    schw = nc.dram_tensor("schw", [128, 2], FP32, kind="ExternalInput")
    wproj = nc.dram_tensor("wproj", [4, 128, 16, 128], BF16, kind="ExternalInput")
    bproj = nc.dram_tensor("bproj", [128, 4], FP32, kind="ExternalInput")
    ident = nc.dram_tensor("ident", [128, 128], FP32, kind="ExternalInput")
    out = nc.dram_tensor("out", [TOK, PROJ_DIM], FP32, kind="ExternalOutput")

    with tile.TileContext(nc) as tc:
        with (
            tc.tile_pool(name="consts", bufs=1) as cpool,
            tc.tile_pool(name="hbuf", bufs=2) as hpool,
            tc.tile_pool(name="wstream", bufs=3) as wpool,
            tc.tile_pool(name="wproj", bufs=2) as wppool,
            tc.tile_pool(name="small", bufs=2) as spool,
        ):
            # xs lives only through the conv phase; its scoped pool frees
            # 50KB for the highway weight stream
            with tc.tile_pool(name="xsp", bufs=1) as xspool:
                # ---- 1+2. gather char embeddings chunk-by-chunk into a
                # small ring buffer; scatter each chunk into the 7 tap bands
                # of the patch matrix as it lands (pipelines gather vs copy).
                with tc.tile_pool(name="gather", bufs=2) as gpool:
                    # idx first on the DMA queue: it gates the gather DGE
                    idx_t = gpool.tile([128, SIdx], mybir.dt.int16, tag="idx")
                    nc.sync.dma_start(out=idx_t[:], in_=idx[:])
                    wconv_t = cpool.tile([KDIM, N_FILTERS], BF16)
                    nc.sync.dma_start(out=wconv_t[:], in_=wconv[:])
                    bconv_t = cpool.tile([128, 16], FP32)
                    nc.sync.dma_start(out=bconv_t[:], in_=bconv[:])
                    bhw_t = cpool.tile([128, 2, 16, 2], FP32)
                    nc.sync.dma_start(out=bhw_t[:], in_=bhw[:])
                    schw_t = cpool.tile([128, 2], FP32)
                    nc.sync.dma_start(out=schw_t[:], in_=schw[:])
                    bproj_t = cpool.tile([128, 4], FP32)
                    nc.sync.dma_start(out=bproj_t[:], in_=bproj[:])
                    ident_t = cpool.tile([128, 128], FP32)
                    nc.sync.dma_start(out=ident_t[:], in_=ident[:])

                    xs = xspool.tile([KDIM, FREE], BF16)
                    # tap band k's last 512k cols correspond to char
                    # positions >= 50 and must read as zero.  One
                    # partition-0-based memset over the union region; the
                    # tap copies below rewrite the valid parts.
                    nc.vector.memset(xs[:, FREE - 512 * 6:], 0)

                    for r in range(NI // GATHER_CHUNK):
                        o = r * GATHER_CHUNK
                        xgc = gpool.tile([128, 1, GATHER_CHUNK], BF16, tag="xgc")
                        nc.gpsimd.dma_gather(
                            out_ap=xgc[:],
                            in_ap=table[:],
                            idxs_ap=idx_t[:, o // 16:(o + GATHER_CHUNK) // 16],
                            num_idxs=GATHER_CHUNK,
                            num_idxs_reg=GATHER_CHUNK,
                            elem_size=128,
                            transpose=True,
                            single_packet=False,
                        )
                        for k in range(7):
                            # xs[16k+d, c] = xg[d, c + 512k]; chunk covers
                            # xg cols [o, o+CHUNK)
                            lo = max(0, o - 512 * k)
                            hi = min(FREE - 512 * k, o + GATHER_CHUNK - 512 * k)
                            if lo >= hi:
                                continue
                            nc.sync.dma_start(
                                out=xs[16 * k:16 * (k + 1), lo:hi],
                                in_=xgc[0:16, 0, lo + 512 * k - o:hi + 512 * k - o],
                            )

                # ---- 3+4. conv + max pool + relu -> h tiles (bf16 + fp8)
                # 2-bank PSUM rounds in a 4-deep ring: drain latency hides
                # behind 3 rounds of PE lookahead.  Per round, greedy pick:
                #  - A2: one 2-bank ACT copy + DVE pair-fold   (ACT 1138/DVE 594)
                #  - D2: ACT copies bank1, DVE maxes bank0 vs it, slab-fold
                #                                              (ACT 712/DVE 986)
                # Four slab chains in acc4 keep folds off the critical path.
                with tc.tile_pool(name="convp", bufs=4, space="PSUM") as convp:
                    h0_b = hpool.tile([128, 16, TOK], BF16, tag="hb")
                    h0_f = hpool.tile([128, 16, TOK], F8, tag="hf")
                    est = {"dve": 0.0, "act": 0.0}
                    for i, spec in enumerate(CH_TILES):
                        lhsT = wconv_t[:, 128 * i:128 * (i + 1)]
                        acc4 = spool.tile([128, 4, TOK], BF16, tag="acc4")
                        first4 = [True, True, True, True]
                        t_cnt = spec["t_main"]
                        ridx = 0
                        t0 = 0
                        while t0 < t_cnt:
                            nt = min(2, t_cnt - t0)
                            P = convp.tile([128, 2, TOK], FP32, tag="ps")
                            for r in range(nt):
                                t = t0 + r
                                nc.tensor.matmul(
                                    out=P[:, r, :], lhsT=lhsT,
                                    rhs=xs[:, TOK * t:TOK * (t + 1)],
                                    start=True, stop=True)
                            if nt == 2:
                                pair = ridx % 2
                                sl = 2 * pair
                                slot = ridx % 4
                                a_first = first4[sl] and first4[sl + 1]
                                # strict D,A,D cycle (f_d=2/3 balances DVE/ACT)
                                # after two forced A2 seed rounds; determinism
                                # avoids greedy burstiness -> steadier pipeline
                                use_a2 = (ridx < 2) or (ridx % 3 == 1)
                                if use_a2:
                                    # A2: 2-bank ACT copy (forced for the
                                    # first two rounds to seed all 4 slots)
                                    est["act"] += 1138.0
                                    if a_first:
                                        nc.scalar.copy(out=acc4[:, sl:sl + 2, :],
                                                       in_=P[:, 0:2, :])
                                        first4[sl] = first4[sl + 1] = False
                                    else:
                                        t2 = spool.tile([128, 2, TOK], BF16,
                                                        tag="sp", name="t2a")
                                        nc.scalar.copy(out=t2[:], in_=P[:, 0:2, :])
                                        nc.vector.tensor_tensor(
                                            out=acc4[:, sl:sl + 2, :],
                                            in0=acc4[:, sl:sl + 2, :],
                                            in1=t2[:], op=MAX_OP)
                                        est["dve"] += 594.0
                                else:
                                    # D2: bank1 via ACT, bank0 via DVE max
                                    c = spool.tile([128, TOK], BF16, tag="c0",
                                                   name="cd")
                                    nc.scalar.copy(out=c[:], in_=P[:, 1, :])
                                    est["act"] += 712.0
                                    if first4[slot]:
                                        nc.vector.tensor_tensor(
                                            out=acc4[:, slot:slot + 1, :],
                                            in0=P[:, 0:1, :],
                                            in1=c[:].unsqueeze(1), op=MAX_OP)
                                        first4[slot] = False
                                        est["dve"] += 658.0
                                    else:
                                        tl = spool.tile([128, 2, TOK], BF16,
                                                        tag="sp", name="tld")
                                        nc.vector.tensor_tensor(
                                            out=tl[:, 0:1, :], in0=P[:, 0:1, :],
                                            in1=c[:].unsqueeze(1), op=MAX_OP)
                                        nc.vector.tensor_tensor(
                                            out=acc4[:, slot:slot + 1, :],
                                            in0=acc4[:, slot:slot + 1, :],
                                            in1=tl[:, 0:1, :], op=MAX_OP)
                                        est["dve"] += 986.0
                            else:
                                # single-bank tail (odd t_cnt)
                                slot = ridx % 4
                                tl = spool.tile([128, 2, TOK], BF16, tag="sp",
                                                name="tl1")
                                nc.scalar.copy(out=tl[:, 0:1, :], in_=P[:, 0:1, :])
                                est["act"] += 712.0
                                nc.vector.tensor_tensor(
                                    out=acc4[:, slot:slot + 1, :],
                                    in0=acc4[:, slot:slot + 1, :],
                                    in1=tl[:, 0:1, :], op=MAX_OP)
                                est["dve"] += 328.0
                            ridx += 1
                            t0 += nt
                        # row-tails (tile 0): positions valid for a row subset
                        if spec["tails"]:
                            P = convp.tile([128, 2, TOK], FP32, tag="ps")
                            for r, (t, hi) in enumerate(spec["tails"]):
                                nc.tensor.matmul(
                                    out=P[:, r, :], lhsT=lhsT,
                                    rhs=xs[:, TOK * t:TOK * (t + 1)],
                                    start=True, stop=True)
                            for r, (t, hi) in enumerate(spec["tails"]):
                                tl = spool.tile([128, 2, TOK], BF16, tag="sp",
                                                name="tlr")
                                nc.scalar.copy(out=tl[:, 0:1, :],
                                               in_=P[:, r:r + 1, :])
                                nc.vector.tensor_tensor(
                                    out=acc4[0:hi, 0:1, :],
                                    in0=acc4[0:hi, 0:1, :],
                                    in1=tl[0:hi, 0:1, :], op=MAX_OP)
                                est["act"] += 712.0
                                est["dve"] += 328.0
                        m2 = spool.tile([128, 2, TOK], BF16, tag="m2")
                        nc.vector.tensor_tensor(
                            out=m2[:], in0=acc4[:, 0:2, :], in1=acc4[:, 2:4, :],
                            op=MAX_OP)
                        v = spool.tile([128, TOK], BF16, tag="vv")
                        nc.vector.tensor_tensor(
                            out=v[:], in0=m2[:, 0, :], in1=m2[:, 1, :],
                            op=MAX_OP)
                        est["dve"] += 594.0 + 328.0
                        nc.scalar.activation(
                            out=h0_b[:, i, :], in_=v[:],
                            func=mybir.ActivationFunctionType.Relu,
                            bias=bconv_t[:, i:i + 1], scale=1.0)
                        est["act"] += 712.0
                        nc.vector.tensor_scalar_mul(
                            out=h0_f[:, i, :], in0=h0_b[:, i, :], scalar1=SH)
                        est["dve"] += 327.0

            # ---- 5. highway layers: fp8 DoubleRow split precision
            DR = mybir.MatmulPerfMode.DoubleRow
            stack = ExitStack()
            hwp = stack.enter_context(tc.tile_pool(name="hwp", bufs=2, space="PSUM"))
            pjp = stack.enter_context(tc.tile_pool(name="pjp", bufs=2, space="PSUM"))
            trp = stack.enter_context(tc.tile_pool(name="trp", bufs=2, space="PSUM"))
            hin_b, hin_f = h0_b, h0_f
            for layer in range(2):
                hout_b = hpool.tile([128, 16, TOK], BF16, tag="hb")
                if layer == 0:
                    hout_f = hpool.tile([128, 16, TOK], F8, tag="hf")
                else:
                    hout_f = None
                for j in range(16):
                    wslab = wpool.tile([128, 2, 2, 8, 2, 128], F8, tag="whw")
                    nc.sync.dma_start(out=wslab[:], in_=whw8[layer, j].rearrange(
                        "p (hl half c s o) -> p hl half c s o",
                        hl=2, half=2, c=8, s=2))
                    Pj = hwp.tile([128, 2, TOK], FP32, tag="pshw", name="pshw")
                    for half in range(2):
                        pdst = Pj[:, half, :]
                        # gate halves: hi chain only -- sigmoid squashes the
                        # ~6% lo-term error (validated within the rel budget)
                        n_hl = 1 if half == 1 else 2
                        for hl in range(n_hl):
                            for c in range(8):
                                nc.tensor.matmul(
                                    out=pdst,
                                    lhsT=wslab[:, hl, half, c],
                                    rhs=hin_f[:, 2 * c:2 * c + 2, :],
                                    start=(hl == 0 and c == 0),
                                    stop=(hl == n_hl - 1 and c == 7),
                                    perf_mode=DR)
                    nl = spool.tile([128, TOK], BF16, tag="nl")
                    gt = spool.tile([128, TOK], BF16, tag="gt")
                    nc.scalar.activation(
                        out=nl[:], in_=Pj[:, 0, :],
                        func=mybir.ActivationFunctionType.Relu,
                        bias=bhw_t[:, layer, j, 0:1],
                        scale=schw_t[:, layer:layer + 1])
                    nc.scalar.activation(
                        out=gt[:], in_=Pj[:, 1, :],
                        func=mybir.ActivationFunctionType.Sigmoid,
                        bias=bhw_t[:, layer, j, 1:2],
                        scale=schw_t[:, layer:layer + 1])
                    d = spool.tile([128, TOK], BF16, tag="d")
                    nc.vector.tensor_tensor(
                        out=d[:], in0=hin_b[:, j, :], in1=nl[:],
                        op=mybir.AluOpType.subtract)
                    nc.vector.tensor_mul(out=d[:], in0=gt[:], in1=d[:])
                    nc.vector.tensor_add(out=hout_b[:, j, :], in0=nl[:], in1=d[:])
                    if hout_f is not None:
                        nc.vector.tensor_scalar_mul(
                            out=hout_f[:, j, :], in0=hout_b[:, j, :], scalar1=SH)
                hin_b, hin_f = hout_b, hout_f

            # ---- 6. projection (bf16) + transpose + out
            for j2 in range(4):
                wp = wppool.tile([128, 16, 128], BF16, tag="wp")
                nc.sync.dma_start(out=wp[:], in_=wproj[j2])
                Pp = pjp.tile([128, TOK], FP32, tag="pspj", name="psproj")
                p_o = Pp[:]
                for c in range(16):
                    nc.tensor.matmul(
                        out=p_o, lhsT=wp[:, c, :], rhs=hin_b[:, c, :],
                        start=(c == 0), stop=(c == 15))
                ot = spool.tile([128, TOK], FP32, tag="ot")
                nc.scalar.activation(
                    out=ot[:], in_=p_o,
                    func=mybir.ActivationFunctionType.Identity,
                    bias=bproj_t[:, j2:j2 + 1], scale=1.0)
                for m4 in range(4):
                    p_t = trp.tile([128, TOK], FP32, tag="pstr", name="pstr")[:, 0:128]
                    nc.tensor.transpose(
                        out=p_t[:], in_=ot[:, 128 * m4:128 * (m4 + 1)],
                        identity=ident_t[:])
                    ob = spool.tile([128, 128], FP32, tag="ob")
                    nc.scalar.copy(out=ob[:], in_=p_t[:])
                    nc.sync.dma_start(
                        out=out[128 * m4:128 * (m4 + 1), 128 * j2:128 * (j2 + 1)],
                        in_=ob[:])
            stack.close()

    nc.compile()
    return nc


_CACHED = {}


def _pow2scale(am, target):
    return 2.0 ** np.floor(np.log2(target / max(am, 1e-20)))


def _prep(inputs):
    """Host-side layout prep: sharding, index arithmetic, weight packing."""
    chars = np.asarray(inputs["chars"]).astype(np.int64).reshape(NTOK, L)

    emb = np.asarray(inputs["char_emb"], np.float32)
    table = np.zeros((PAD_V, 128), np.float32)
    table[:CHAR_VOCAB, :CHAR_DIM] = emb
    table = table.astype(ml_dtypes.bfloat16)

    # conv weights -> (112, 2048) zero-padded taps, matching X_stack rows 16k+d
    wc = np.zeros((7, CHAR_DIM, N_FILTERS), np.float32)
    off = 0
    for fi, (w, n) in enumerate(FILTERS):
        cw = np.asarray(inputs[f"conv_w_{fi}"], np.float32)  # (n, 16, w)
        wc[:w, :, off:off + n] = cw.transpose(2, 1, 0)
        off += n
    wconv = wc.reshape(KDIM, N_FILTERS).astype(ml_dtypes.bfloat16)
    bconv = np.concatenate([np.asarray(inputs[f"conv_b_{i}"], np.float32)
                            for i in range(7)])
    bconv_dev = bconv.reshape(16, 128).T.copy()  # (128, 16)

    # highway weights: fp8 split, DoubleRow layout
    # whw8[l, j, p, (hl, half, c, s2, o)], value = Wsplit[hl][ic, col]
    # with ic = 256c + 128*s2 + p, col = 2048*half + 128j + o
    whw8 = np.zeros((2, 16, 128, 8192), ml_dtypes.float8_e4m3)
    bhw = np.zeros((128, 2, 16, 2), np.float32)
    schw = np.zeros((128, 2), np.float32)
    for l in range(2):
        W = np.asarray(inputs[f"hw_w_{l}"], np.float32)   # (4096, 2048)
        bb = np.asarray(inputs[f"hw_b_{l}"], np.float32)  # (4096,)
        WT = W.T  # (2048, 4096)
        s = _pow2scale(np.abs(WT).max(), 120.0)
        Whi = (WT * s).astype(ml_dtypes.float8_e4m3)
        Wlo = (WT * s - Whi.astype(np.float32)).astype(ml_dtypes.float8_e4m3)
        A = np.stack([Whi, Wlo]).astype(np.float32)       # (hl, 2048, 4096)
        A = A.reshape(2, 8, 2, 128, 2, 16, 128)           # hl c s2 p half j o
        A = A.transpose(5, 3, 0, 4, 1, 2, 6)              # j p hl half c s2 o
        whw8[l] = A.reshape(16, 128, 8192).astype(ml_dtypes.float8_e4m3)
        schw[:, l] = 1.0 / (s * SH)
        for j in range(16):
            bhw[:, l, j, 0] = bb[128 * j:128 * (j + 1)]
            bhw[:, l, j, 1] = bb[2048 + 128 * j:2048 + 128 * (j + 1)]

    Wp = np.asarray(inputs["proj_w"], np.float32)  # (512, 2048)
    WpT = Wp.T  # (2048, 512)
    # wproj[j2, p, c, o] = WpT[128c + p, 128j2 + o]
    wproj = WpT.reshape(16, 128, 4, 128).transpose(2, 1, 0, 3).astype(
        ml_dtypes.bfloat16).copy()
    bproj = np.zeros((128, 4), np.float32)
    bp = np.asarray(inputs["proj_b"], np.float32)
    for j2 in range(4):
        bproj[:, j2] = bp[128 * j2:128 * (j2 + 1)]

    ident = np.eye(128, dtype=np.float32)

    shared = dict(table=table, wconv=wconv, bconv=bconv_dev, whw8=whw8,
                  bhw=bhw, schw=schw, wproj=wproj, bproj=bproj, ident=ident)

    in_maps = []
    for core in range(N_CORES):
        cp = chars[core * TOK:(core + 1) * TOK]  # (512, 50)
        # flat index j = t'*512 + n  ->  idx_flat[j] = cp[n, t']
        idx_flat = cp.T.reshape(-1).astype(np.int16)  # (25600,)
        idx16 = idx_flat.reshape(NI // 16, 16).T.copy()  # (16, S)
        idx16 = np.tile(idx16, (8, 1))  # (128, S)
        m = dict(shared)
        m["idx"] = idx16
        in_maps.append(m)
    return in_maps


def kernel(**inputs) -> np.ndarray:
    if "nc" not in _CACHED:
        _CACHED["nc"] = build_module()
    nc = _CACHED["nc"]
    in_maps = _prep(inputs)
    res = run_bass_kernel_spmd(nc, in_maps, core_ids=list(range(N_CORES)))
    full = np.concatenate([r["out"] for r in res.results], axis=0)
    return full.reshape(B, S, PROJ_DIM)


# revision 71
# speedup vs baseline: 1.0475x; 1.0160x over previous
"""CharCNN token embedder (ELMo-style) on 8 Trainium2 NeuronCores.

Data-parallel over the 4096 = 16*256 tokens (512 per core). Weights replicated.

Per-core pipeline:
  1. dma_gather (transpose mode) pulls char-embedding rows (padded to 256B)
     into feature-major layout X[d, (t', n)] for t' in [0,56), n in [0,512).
  2. 7 shifted SBUF->SBUF copies build the patch matrix Xs[(k,d), (t, n)]
     (112 x 25600) = im2col for a width-7 window (weights zero-padded).
  3. Conv = bf16 matmuls with K=112: per 128-channel tile, one matmul per
     valid position t (N=512 tokens), PSUM rounds of 4 banks.
  4. Max-pool over positions per 4-bank PSUM round: ACT copies banks 1,3
     to SBUF, DVE pair-maxes them against banks 0,2 (one PSUM operand per
     DVE op -- a walrus requirement), then folds into two alternating
     accumulator chains; relu+bias at tile finalize; h written both bf16
     and fp8 (x64 scale).
  5. 2 highway layers in fp8e4 DoubleRow split precision: W ~ (Whi+Wlo)/s,
     K=256 per DR matmul, 16 DR matmuls per (j, half) accumulate one PSUM
     bank; ACT applies 1/(s*64) scale (per-partition AP) + bias + relu /
     sigmoid; DVE does the highway gating in bf16 4x mode.
  6. Projection to 512 in bf16, bias, PE-transpose to token-major, DMA out.
"""

from contextlib import ExitStack

import numpy as np
import ml_dtypes

import concourse.bass as bass
import concourse.mybir as mybir
import concourse.tile as tile
from concourse import bacc
from concourse.bass_utils import run_bass_kernel_spmd
from concourse.vector_clock import ScopedClock

# ---------------------------------------------------------------- constants
B, S, L = 16, 256, 50
CHAR_DIM = 16
CHAR_VOCAB = 262
PAD_V = 264            # table rows (262 real + 1 zero pad row + 1 spare)
ZERO_ROW = 262
FILTERS = [(1, 32), (2, 32), (3, 64), (4, 128), (5, 256), (6, 512), (7, 1024)]
N_FILTERS = 2048
PROJ_DIM = 512
N_CORES = 8
NTOK = B * S                  # 4096
TOK = NTOK // N_CORES         # 512 tokens per core
NPOS = 50                     # conv output positions computed
NI = TOK * NPOS               # gather indices per core = 25600
GATHER_CHUNK = 6400           # indices per dma_gather (descriptor-ring safe)
FREE = TOK * NPOS             # X_stack free size = 25600
KDIM = 112                    # 7 taps * 16 dims
SH = 64.0                     # fp8 activation scale for h0/h1 (absmax ~0.65)

# per 128-channel tile: full-row position count + (t, row_hi) row-tails
CH_TILES = [{"t_main": 48, "tails": [(48, 64), (49, 32)]}]
CH_TILES.append({"t_main": 47, "tails": []})      # w4
for _ in range(2):
    CH_TILES.append({"t_main": 46, "tails": []})  # w5
for _ in range(4):
    CH_TILES.append({"t_main": 45, "tails": []})  # w6
for _ in range(8):
    CH_TILES.append({"t_main": 44, "tails": []})  # w7
R_POS = 4

BF16 = mybir.dt.bfloat16
FP32 = mybir.dt.float32
F8 = mybir.dt.float8e4
MAX_OP = mybir.AluOpType.max

_MAX_WAITS_PER_INST = 1


def _patched_drain_and_barrier(self, tick_clock, wait_clock):
    # The walrus build in this container rejects CTRL instructions carrying
    # more than one sem wait; spread the kernel-tail drain waits over NOPs.
    nc = self.nc
    carrier = nc.sync.nop()
    wait_clock.add_sem_waits(carrier.ins, ScopedClock({None: tick_clock.global_clock}))
    si = carrier.ins.sync_info
    waits = list(si.on_wait) if si is not None and si.on_wait else []
    if len(waits) > _MAX_WAITS_PER_INST:
        carrier.ins.sync_info = mybir.SyncInfo(
            on_wait=waits[:_MAX_WAITS_PER_INST],
            on_update=list(si.on_update) if si.on_update else [])
        for i in range(_MAX_WAITS_PER_INST, len(waits), _MAX_WAITS_PER_INST):
            extra = nc.sync.nop()
            extra.ins.sync_info = mybir.SyncInfo(
                on_wait=waits[i:i + _MAX_WAITS_PER_INST], on_update=[])
    nc.sync.drain()
    nc.all_engine_barrier()
    assert self.sems is not None
    popped = nc._tile_sem_poison_stack.pop()
    assert popped is self._sem_poison
    nc.clear_and_free_semaphores(list(self.sems.allocated().values()))
    nc.all_engine_barrier()


tile.TileContext._drain_and_barrier = _patched_drain_and_barrier


class PoolSched:
    """Greedy engine-load balancer for the conv max-pool stage.

    Cost constants are exact TimelineSim per-instruction engine times."""

    def __init__(self, nc, spool):
        self.nc = nc
        self.spool = spool
        self.est = {"dve": 0.0, "act": 0.0, "pool": 0.0}

    def _pick(self, options):
        """options: list of (key, {eng: cost}). Pick min resulting max-load."""
        best, bestv = None, None
        for key, costs in options:
            peak = max(self.est[e] + costs.get(e, 0.0) for e in self.est)
            if bestv is None or peak < bestv:
                best, bestv = (key, costs), peak
        for e, c in best[1].items():
            self.est[e] += c
        return best[0]

    def fold(self, acc_ap, in_ap, nelem):
        """acc = max(acc, in); DVE only (walrus rejects TT on gpsimd)."""
        self.est["dve"] += {512: 328.0, 1024: 594.0, 2048: 1127.0}[nelem]
        self.nc.vector.tensor_tensor(
            out=acc_ap, in0=acc_ap, in1=in_ap, op=MAX_OP)

    def round(self, P, nt, st, ridx):
        """Drain one PSUM round (nt banks) into an independent chain.

        walrus allows at most ONE PSUM operand per DVE op, so drains are
        either a fused TT(PSUM, acc_sbuf)->acc (quad chains D1/D2 on DVE)
        or an ACT copy + SBUF-side fold (quad chain C).  acc16 slots:
        0:4 = D1, 4:8 = D2, 8:12 = C."""
        nc, spool = self.nc, self.spool
        acc = st["acc16"]
        if nt == 4:
            path = self._pick([("dve", {"dve": 2258.0}),
                               ("act", {"act": 1992.0, "dve": 0.0})])
            if path == "dve":
                k = st["dsel"]
                st["dsel"] ^= 1
                sl = 4 * k
                if st["first"][k]:
                    nc.vector.tensor_copy(out=acc[:, sl:sl + 4, :], in_=P[:, 0:4, :])
                    st["first"][k] = False
                else:
                    nc.vector.tensor_tensor(
                        out=acc[:, sl:sl + 4, :], in0=P[:, 0:4, :],
                        in1=acc[:, sl:sl + 4, :], op=MAX_OP)
            else:
                k = st["csel"]
                st["csel"] ^= 1
                sl = 8 + 4 * k
                if st["first"][2 + k]:
                    nc.scalar.copy(out=acc[:, sl:sl + 4, :], in_=P[:, 0:4, :])
                    st["first"][2 + k] = False
                else:
                    t = spool.tile([128, 4, TOK], BF16, tag="t4", name="tdr4")
                    nc.scalar.copy(out=t[:], in_=P[:, 0:4, :])
                    self.fold(acc[:, sl:sl + 4, :], t[:], 2048)
        else:
            # tail rounds (1-3 banks): fused max into chain D1's prefix
            if st["first"][0]:
                nc.vector.tensor_copy(out=acc[:, 0:nt, :], in_=P[:, 0:nt, :])
                st["first"][0] = False
                # remaining D1 slots stay virgin: seed them too
                if nt < 4:
                    nc.vector.tensor_copy(out=acc[:, nt:4, :],
                                          in_=P[:, 0:4 - nt, :])
            else:
                nc.vector.tensor_tensor(
                    out=acc[:, 0:nt, :], in0=P[:, 0:nt, :],
                    in1=acc[:, 0:nt, :], op=MAX_OP)
            self.est["dve"] += {1: 658.0, 2: 1192.0, 3: 1725.0}[nt]

    def finalize(self, st, h_b, h_f, bias_ap, i):
        nc, spool = self.nc, self.spool
        acc = st["acc16"]
        # combine quad chains (DVE)
        srcs = [acc[:, 0:4, :]]
        for k in (1, 2, 3):
            if not st["first"][k]:
                srcs.append(acc[:, 4 * k:4 * k + 4, :])
        while len(srcs) > 1:
            b = srcs.pop()
            a = srcs[-1]
            self.est["dve"] += 1127.0
            nc.vector.tensor_tensor(out=a, in0=a, in1=b, op=MAX_OP)
        m2 = spool.tile([128, 2, TOK], BF16, tag="t2")
        nc.vector.tensor_tensor(
            out=m2[:], in0=acc[:, 0:2, :], in1=acc[:, 2:4, :], op=MAX_OP)
        nc.vector.tensor_tensor(
            out=m2[:, 0:1, :], in0=m2[:, 0:1, :], in1=m2[:, 1:2, :], op=MAX_OP)
        self.est["dve"] += 594.0 + 328.0
        nc.scalar.activation(
            out=h_b[:, i, :], in_=m2[:, 0, :],
            func=mybir.ActivationFunctionType.Relu,
            bias=bias_ap, scale=1.0)
        self.est["act"] += 712.0
        if h_f is not None:
            self.est["dve"] += 327.0
            nc.vector.tensor_scalar_mul(
                out=h_f[:, i, :], in0=h_b[:, i, :], scalar1=SH)


# ---------------------------------------------------------------- device IR
def build_module():
    nc = bacc.Bacc()
    SIdx = NI // 16  # 1792 int16 columns

    table = nc.dram_tensor("table", [PAD_V, 128], BF16, kind="ExternalInput")
    idx = nc.dram_tensor("idx", [128, SIdx], mybir.dt.int16, kind="ExternalInput")
    wconv = nc.dram_tensor("wconv", [KDIM, N_FILTERS], BF16, kind="ExternalInput")
    bconv = nc.dram_tensor("bconv", [128, 16], FP32, kind="ExternalInput")
    # fp8 split highway weights: [l, j, p, (hl, half, c, s2, o)] flattened
    whw8 = nc.dram_tensor("whw8", [2, 16, 128, 6144], F8, kind="ExternalInput")
    bhw = nc.dram_tensor("bhw", [128, 2, 16, 2], FP32, kind="ExternalInput")
    schw = nc.dram_tensor("schw", [128, 2], FP32, kind="ExternalInput")
    wproj = nc.dram_tensor("wproj", [4, 128, 16, 128], BF16, kind="ExternalInput")
    bproj = nc.dram_tensor("bproj", [128, 4], FP32, kind="ExternalInput")
    ident = nc.dram_tensor("ident", [128, 128], FP32, kind="ExternalInput")
    out = nc.dram_tensor("out", [TOK, PROJ_DIM], FP32, kind="ExternalOutput")

    with tile.TileContext(nc) as tc:
        with (
            tc.tile_pool(name="consts", bufs=1) as cpool,
            tc.tile_pool(name="hbuf", bufs=2) as hpool,
            tc.tile_pool(name="wstream", bufs=3) as wpool,
            tc.tile_pool(name="wproj", bufs=2) as wppool,
            tc.tile_pool(name="small", bufs=2) as spool,
        ):
            # xs lives only through the conv phase; its scoped pool frees
            # 50KB for the highway weight stream
            with tc.tile_pool(name="xsp", bufs=1) as xspool:
                # ---- 1+2. gather char embeddings chunk-by-chunk into a
                # small ring buffer; scatter each chunk into the 7 tap bands
                # of the patch matrix as it lands (pipelines gather vs copy).
                with tc.tile_pool(name="gather", bufs=2) as gpool:
                    # idx first on the DMA queue: it gates the gather DGE
                    idx_t = gpool.tile([128, SIdx], mybir.dt.int16, tag="idx")
                    nc.sync.dma_start(out=idx_t[:], in_=idx[:])
                    wconv_t = cpool.tile([KDIM, N_FILTERS], BF16)
                    nc.sync.dma_start(out=wconv_t[:], in_=wconv[:])
                    bconv_t = cpool.tile([128, 16], FP32)
                    nc.sync.dma_start(out=bconv_t[:], in_=bconv[:])
                    bhw_t = cpool.tile([128, 2, 16, 2], FP32)
                    nc.sync.dma_start(out=bhw_t[:], in_=bhw[:])
                    schw_t = cpool.tile([128, 2], FP32)
                    nc.sync.dma_start(out=schw_t[:], in_=schw[:])
                    bproj_t = cpool.tile([128, 4], FP32)
                    nc.sync.dma_start(out=bproj_t[:], in_=bproj[:])
                    ident_t = cpool.tile([128, 128], FP32)
                    nc.sync.dma_start(out=ident_t[:], in_=ident[:])

                    xs = xspool.tile([KDIM, FREE], BF16)
                    # tap band k's last 512k cols correspond to char
                    # positions >= 50 and must read as zero.  One
                    # partition-0-based memset over the union region; the
                    # tap copies below rewrite the valid parts.
                    nc.vector.memset(xs[:, FREE - 512 * 6:], 0)

                    for r in range(NI // GATHER_CHUNK):
                        o = r * GATHER_CHUNK
                        xgc = gpool.tile([128, 1, GATHER_CHUNK], BF16, tag="xgc")
                        nc.gpsimd.dma_gather(
                            out_ap=xgc[:],
                            in_ap=table[:],
                            idxs_ap=idx_t[:, o // 16:(o + GATHER_CHUNK) // 16],
                            num_idxs=GATHER_CHUNK,
                            num_idxs_reg=GATHER_CHUNK,
                            elem_size=128,
                            transpose=True,
                            single_packet=False,
                        )
                        for k in range(7):
                            # xs[16k+d, c] = xg[d, c + 512k]; chunk covers
                            # xg cols [o, o+CHUNK)
                            lo = max(0, o - 512 * k)
                            hi = min(FREE - 512 * k, o + GATHER_CHUNK - 512 * k)
                            if lo >= hi:
                                continue
                            nc.sync.dma_start(
                                out=xs[16 * k:16 * (k + 1), lo:hi],
                                in_=xgc[0:16, 0, lo + 512 * k - o:hi + 512 * k - o],
                            )

                # ---- 3+4. conv + max pool + relu -> h tiles (bf16 + fp8)
                # 2-bank PSUM rounds in a 4-deep ring: drain latency hides
                # behind 3 rounds of PE lookahead.  Per round, greedy pick:
                #  - A2: one 2-bank ACT copy + DVE pair-fold   (ACT 1138/DVE 594)
                #  - D2: ACT copies bank1, DVE maxes bank0 vs it, slab-fold
                #                                              (ACT 712/DVE 986)
                # Four slab chains in acc4 keep folds off the critical path.
                with tc.tile_pool(name="convp", bufs=4, space="PSUM") as convp:
                    h0_b = hpool.tile([128, 16, TOK], BF16, tag="hb")
                    h0_f = hpool.tile([128, 16, TOK], F8, tag="hf")
                    est = {"dve": 0.0, "act": 0.0}
                    for i, spec in enumerate(CH_TILES):
                        lhsT = wconv_t[:, 128 * i:128 * (i + 1)]
                        acc4 = spool.tile([128, 4, TOK], BF16, tag="acc4")
                        first4 = [True, True, True, True]
                        t_cnt = spec["t_main"]
                        ridx = 0
                        t0 = 0
                        while t0 < t_cnt:
                            nt = min(2, t_cnt - t0)
                            P = convp.tile([128, 2, TOK], FP32, tag="ps")
                            for r in range(nt):
                                t = t0 + r
                                nc.tensor.matmul(
                                    out=P[:, r, :], lhsT=lhsT,
                                    rhs=xs[:, TOK * t:TOK * (t + 1)],
                                    start=True, stop=True)
                            if nt == 2:
                                pair = ridx % 2
                                sl = 2 * pair
                                slot = ridx % 4
                                a_first = first4[sl] and first4[sl + 1]
                                # strict D,A,D cycle (f_d=2/3 balances DVE/ACT)
                                # after two forced A2 seed rounds; determinism
                                # avoids greedy burstiness -> steadier pipeline
                                use_a2 = (ridx < 2) or (ridx % 3 == 1)
                                if use_a2:
                                    # A2: 2-bank ACT copy (forced for the
                                    # first two rounds to seed all 4 slots)
                                    est["act"] += 1138.0
                                    if a_first:
                                        nc.scalar.copy(out=acc4[:, sl:sl + 2, :],
                                                       in_=P[:, 0:2, :])
                                        first4[sl] = first4[sl + 1] = False
                                    else:
                                        t2 = spool.tile([128, 2, TOK], BF16,
                                                        tag="sp", name="t2a")
                                        nc.scalar.copy(out=t2[:], in_=P[:, 0:2, :])
                                        nc.vector.tensor_tensor(
                                            out=acc4[:, sl:sl + 2, :],
                                            in0=acc4[:, sl:sl + 2, :],
                                            in1=t2[:], op=MAX_OP)
                                        est["dve"] += 594.0
                                else:
                                    # D2: bank1 via ACT, bank0 via DVE max
                                    c = spool.tile([128, TOK], BF16, tag="c0",
                                                   name="cd")
                                    nc.scalar.copy(out=c[:], in_=P[:, 1, :])
                                    est["act"] += 712.0
                                    if first4[slot]:
                                        nc.vector.tensor_tensor(
                                            out=acc4[:, slot:slot + 1, :],
                                            in0=P[:, 0:1, :],
                                            in1=c[:].unsqueeze(1), op=MAX_OP)
                                        first4[slot] = False
                                        est["dve"] += 658.0
                                    else:
                                        tl = spool.tile([128, 2, TOK], BF16,
                                                        tag="sp", name="tld")
                                        nc.vector.tensor_tensor(
                                            out=tl[:, 0:1, :], in0=P[:, 0:1, :],
                                            in1=c[:].unsqueeze(1), op=MAX_OP)
                                        nc.vector.tensor_tensor(
                                            out=acc4[:, slot:slot + 1, :],
                                            in0=acc4[:, slot:slot + 1, :],
                                            in1=tl[:, 0:1, :], op=MAX_OP)
                                        est["dve"] += 986.0
                            else:
                                # single-bank tail (odd t_cnt)
                                slot = ridx % 4
                                tl = spool.tile([128, 2, TOK], BF16, tag="sp",
                                                name="tl1")
                                nc.scalar.copy(out=tl[:, 0:1, :], in_=P[:, 0:1, :])
                                est["act"] += 712.0
                                nc.vector.tensor_tensor(
                                    out=acc4[:, slot:slot + 1, :],
                                    in0=acc4[:, slot:slot + 1, :],
                                    in1=tl[:, 0:1, :], op=MAX_OP)
                                est["dve"] += 328.0
                            ridx += 1
                            t0 += nt
                        # row-tails (tile 0): positions valid for a row subset
                        if spec["tails"]:
                            P = convp.tile([128, 2, TOK], FP32, tag="ps")
                            for r, (t, hi) in enumerate(spec["tails"]):
                                nc.tensor.matmul(
                                    out=P[:, r, :], lhsT=lhsT,
                                    rhs=xs[:, TOK * t:TOK * (t + 1)],
                                    start=True, stop=True)
                            for r, (t, hi) in enumerate(spec["tails"]):
                                tl = spool.tile([128, 2, TOK], BF16, tag="sp",
                                                name="tlr")
                                nc.scalar.copy(out=tl[:, 0:1, :],
                                               in_=P[:, r:r + 1, :])
                                nc.vector.tensor_tensor(
                                    out=acc4[0:hi, 0:1, :],
                                    in0=acc4[0:hi, 0:1, :],
                                    in1=tl[0:hi, 0:1, :], op=MAX_OP)
                                est["act"] += 712.0
                                est["dve"] += 328.0
                        m2 = spool.tile([128, 2, TOK], BF16, tag="m2")
                        nc.vector.tensor_tensor(
                            out=m2[:], in0=acc4[:, 0:2, :], in1=acc4[:, 2:4, :],
                            op=MAX_OP)
                        v = spool.tile([128, TOK], BF16, tag="vv")
                        nc.vector.tensor_tensor(
                            out=v[:], in0=m2[:, 0, :], in1=m2[:, 1, :],
                            op=MAX_OP)
                        est["dve"] += 594.0 + 328.0
                        nc.scalar.activation(
                            out=h0_b[:, i, :], in_=v[:],
                            func=mybir.ActivationFunctionType.Relu,
                            bias=bconv_t[:, i:i + 1], scale=1.0)
                        est["act"] += 712.0
                        nc.vector.tensor_scalar_mul(
                            out=h0_f[:, i, :], in0=h0_b[:, i, :], scalar1=SH)
                        est["dve"] += 327.0

            # ---- 5. highway layers: fp8 DoubleRow split precision
            DR = mybir.MatmulPerfMode.DoubleRow
            stack = ExitStack()
            hwp = stack.enter_context(tc.tile_pool(name="hwp", bufs=2, space="PSUM"))
            pjp = stack.enter_context(tc.tile_pool(name="pjp", bufs=2, space="PSUM"))
            trp = stack.enter_context(tc.tile_pool(name="trp", bufs=2, space="PSUM"))
            hin_b, hin_f = h0_b, h0_f
            for layer in range(2):
                hout_b = hpool.tile([128, 16, TOK], BF16, tag="hb")
                if layer == 0:
                    hout_f = hpool.tile([128, 16, TOK], F8, tag="hf")
                else:
                    hout_f = None
                for j in range(16):
                    wslab = wpool.tile([128, 3, 8, 2, 128], F8, tag="whw")
                    nc.sync.dma_start(out=wslab[:], in_=whw8[layer, j].rearrange(
                        "p (b c s o) -> p b c s o", b=3, c=8, s=2))
                    Pj = hwp.tile([128, 2, TOK], FP32, tag="pshw", name="pshw")
                    for half in range(2):
                        pdst = Pj[:, half, :]
                        # blocks: 0 = nl-hi, 1 = gate-hi, 2 = nl-lo; the gate
                        # half is hi-only (sigmoid squashes the ~6% lo-term;
                        # validated) so its lo slab is not even shipped
                        blocks = (0, 2) if half == 0 else (1,)
                        for bi, blk in enumerate(blocks):
                            for c in range(8):
                                nc.tensor.matmul(
                                    out=pdst,
                                    lhsT=wslab[:, blk, c],
                                    rhs=hin_f[:, 2 * c:2 * c + 2, :],
                                    start=(bi == 0 and c == 0),
                                    stop=(bi == len(blocks) - 1 and c == 7),
                                    perf_mode=DR)
                    nl = spool.tile([128, TOK], BF16, tag="nl")
                    gt = spool.tile([128, TOK], BF16, tag="gt")
                    nc.scalar.activation(
                        out=nl[:], in_=Pj[:, 0, :],
                        func=mybir.ActivationFunctionType.Relu,
                        bias=bhw_t[:, layer, j, 0:1],
                        scale=schw_t[:, layer:layer + 1])
                    nc.scalar.activation(
                        out=gt[:], in_=Pj[:, 1, :],
                        func=mybir.ActivationFunctionType.Sigmoid,
                        bias=bhw_t[:, layer, j, 1:2],
                        scale=schw_t[:, layer:layer + 1])
                    d = spool.tile([128, TOK], BF16, tag="d")
                    nc.vector.tensor_tensor(
                        out=d[:], in0=hin_b[:, j, :], in1=nl[:],
                        op=mybir.AluOpType.subtract)
                    nc.vector.tensor_mul(out=d[:], in0=gt[:], in1=d[:])
                    nc.vector.tensor_add(out=hout_b[:, j, :], in0=nl[:], in1=d[:])
                    if hout_f is not None:
                        nc.vector.tensor_scalar_mul(
                            out=hout_f[:, j, :], in0=hout_b[:, j, :], scalar1=SH)
                hin_b, hin_f = hout_b, hout_f

            # ---- 6. projection (bf16) + transpose + out
            for j2 in range(4):
                wp = wppool.tile([128, 16, 128], BF16, tag="wp")
                nc.sync.dma_start(out=wp[:], in_=wproj[j2])
                Pp = pjp.tile([128, TOK], FP32, tag="pspj", name="psproj")
                p_o = Pp[:]
                for c in range(16):
                    nc.tensor.matmul(
                        out=p_o, lhsT=wp[:, c, :], rhs=hin_b[:, c, :],
                        start=(c == 0), stop=(c == 15))
                ot = spool.tile([128, TOK], FP32, tag="ot")
                nc.scalar.activation(
                    out=ot[:], in_=p_o,
                    func=mybir.ActivationFunctionType.Identity,
                    bias=bproj_t[:, j2:j2 + 1], scale=1.0)
                for m4 in range(4):
                    p_t = trp.tile([128, TOK], FP32, tag="pstr", name="pstr")[:, 0:128]
                    nc.tensor.transpose(
                        out=p_t[:], in_=ot[:, 128 * m4:128 * (m4 + 1)],
                        identity=ident_t[:])
                    ob = spool.tile([128, 128], FP32, tag="ob")
                    nc.scalar.copy(out=ob[:], in_=p_t[:])
                    nc.sync.dma_start(
                        out=out[128 * m4:128 * (m4 + 1), 128 * j2:128 * (j2 + 1)],
                        in_=ob[:])
            stack.close()

    nc.compile()
    return nc


_CACHED = {}


def _pow2scale(am, target):
    return 2.0 ** np.floor(np.log2(target / max(am, 1e-20)))


def _prep(inputs):
    """Host-side layout prep: sharding, index arithmetic, weight packing."""
    chars = np.asarray(inputs["chars"]).astype(np.int64).reshape(NTOK, L)

    emb = np.asarray(inputs["char_emb"], np.float32)
    table = np.zeros((PAD_V, 128), np.float32)
    table[:CHAR_VOCAB, :CHAR_DIM] = emb
    table = table.astype(ml_dtypes.bfloat16)

    # conv weights -> (112, 2048) zero-padded taps, matching X_stack rows 16k+d
    wc = np.zeros((7, CHAR_DIM, N_FILTERS), np.float32)
    off = 0
    for fi, (w, n) in enumerate(FILTERS):
        cw = np.asarray(inputs[f"conv_w_{fi}"], np.float32)  # (n, 16, w)
        wc[:w, :, off:off + n] = cw.transpose(2, 1, 0)
        off += n
    wconv = wc.reshape(KDIM, N_FILTERS).astype(ml_dtypes.bfloat16)
    bconv = np.concatenate([np.asarray(inputs[f"conv_b_{i}"], np.float32)
                            for i in range(7)])
    bconv_dev = bconv.reshape(16, 128).T.copy()  # (128, 16)

    # highway weights: fp8 split, DoubleRow layout
    # whw8[l, j, p, (hl, half, c, s2, o)], value = Wsplit[hl][ic, col]
    # with ic = 256c + 128*s2 + p, col = 2048*half + 128j + o
    whw8 = np.zeros((2, 16, 128, 6144), ml_dtypes.float8_e4m3)
    bhw = np.zeros((128, 2, 16, 2), np.float32)
    schw = np.zeros((128, 2), np.float32)
    for l in range(2):
        W = np.asarray(inputs[f"hw_w_{l}"], np.float32)   # (4096, 2048)
        bb = np.asarray(inputs[f"hw_b_{l}"], np.float32)  # (4096,)
        WT = W.T  # (2048, 4096)
        s = _pow2scale(np.abs(WT).max(), 120.0)
        Whi = (WT * s).astype(ml_dtypes.float8_e4m3)
        Wlo = (WT * s - Whi.astype(np.float32)).astype(ml_dtypes.float8_e4m3)
        A = np.stack([Whi, Wlo]).astype(np.float32)       # (hl, 2048, 4096)
        A = A.reshape(2, 8, 2, 128, 2, 16, 128)           # hl c s2 p half j o
        A = A.transpose(5, 3, 0, 4, 1, 2, 6)              # j p hl half c s2 o
        # blocks: nl-hi, gate-hi, nl-lo (gate-lo unused, not shipped)
        A = np.stack([A[:, :, 0, 0], A[:, :, 0, 1], A[:, :, 1, 0]], axis=2)
        whw8[l] = A.reshape(16, 128, 6144).astype(ml_dtypes.float8_e4m3)
        schw[:, l] = 1.0 / (s * SH)
        for j in range(16):
            bhw[:, l, j, 0] = bb[128 * j:128 * (j + 1)]
            bhw[:, l, j, 1] = bb[2048 + 128 * j:2048 + 128 * (j + 1)]

    Wp = np.asarray(inputs["proj_w"], np.float32)  # (512, 2048)
    WpT = Wp.T  # (2048, 512)
    # wproj[j2, p, c, o] = WpT[128c + p, 128j2 + o]
    wproj = WpT.reshape(16, 128, 4, 128).transpose(2, 1, 0, 3).astype(
        ml_dtypes.bfloat16).copy()
    bproj = np.zeros((128, 4), np.float32)
    bp = np.asarray(inputs["proj_b"], np.float32)
    for j2 in range(4):
        bproj[:, j2] = bp[128 * j2:128 * (j2 + 1)]

    ident = np.eye(128, dtype=np.float32)

    shared = dict(table=table, wconv=wconv, bconv=bconv_dev, whw8=whw8,
                  bhw=bhw, schw=schw, wproj=wproj, bproj=bproj, ident=ident)

    in_maps = []
    for core in range(N_CORES):
        cp = chars[core * TOK:(core + 1) * TOK]  # (512, 50)
        # flat index j = t'*512 + n  ->  idx_flat[j] = cp[n, t']
        idx_flat = cp.T.reshape(-1).astype(np.int16)  # (25600,)
        idx16 = idx_flat.reshape(NI // 16, 16).T.copy()  # (16, S)
        idx16 = np.tile(idx16, (8, 1))  # (128, S)
        m = dict(shared)
        m["idx"] = idx16
        in_maps.append(m)
    return in_maps


def kernel(**inputs) -> np.ndarray:
    if "nc" not in _CACHED:
        _CACHED["nc"] = build_module()
    nc = _CACHED["nc"]
    in_maps = _prep(inputs)
    res = run_bass_kernel_spmd(nc, in_maps, core_ids=list(range(N_CORES)))
    full = np.concatenate([r["out"] for r in res.results], axis=0)
    return full.reshape(B, S, PROJ_DIM)
